# revision 2
# baseline (speedup 1.0000x reference)
"""Cached multi-head attention on 8 TRN2 NeuronCores.

Sharding: core c = 2*b + g handles batch b (of 4) and head-group g (of 2,
8 heads each) -- data parallel on batch x tensor parallel on heads.
Column-parallel Wq/Wk/Wv, row-parallel Wo; the Wo all-reduce (sum of the
two head-group partials per batch) is done on host during the unshard,
along with the bo bias add.

Device layout (per core), all matmuls in float32r (full PE rate):
  xT = x.T in HBM (host pre-transposed). Projections:
    qT[d,t] = sum_c WqT[c,d] xqT[c,t]  (+bq)   -> SBUF pair tiles [128, T]
    kT likewise; v[s,d] = sum_c xvT[c,s] WvT[c,d] (+bv via K=1 ones matmul)
  Attention per head-pair (2 heads row-packed in the 128-partition dim):
    ST[s,t] = kT.T @ qT   (K=64 row-tiled, both heads concurrent)
    P = exp(ST/8)         (ScalarE, free scale; no max-subtract needed --
                           scores are O(1) by construction)
    oT_aug = [V|1].T @ P  (K=128, M=65; row 64 = softmax denominators)
    o = oT * (1/denom)    (DVE mult with gpsimd-broadcast reciprocal)
  Out-projection: out[t,e] = sum_d oT[d,t] WoT[d,e], accumulated over the
  4 pair-chunks of d.

Causal masks get a fast path: blocks above the diagonal are skipped,
diagonal blocks use shortened matmuls + gpsimd affine_select zeroing.
Arbitrary masks fall back to per-block skip/plain/mixed classification
with host-shipped multiplicative mask tiles.
"""

import math
import ml_dtypes
import numpy as np

import concourse.bass as bass
import concourse.mybir as mybir
import concourse.tile as tile
from concourse import bacc
from concourse.bass_utils import run_bass_kernel_spmd

F32 = mybir.dt.float32
F32R = mybir.dt.float32r
BF16 = mybir.dt.bfloat16
AF = mybir.ActivationFunctionType
ts = bass.ts

B, T, D, H = 4, 2048, 1024, 16
HD = D // H          # 64
NCORE = 8
DG = D // 2          # 512 dims per core (8 heads)
NPAIR = 4            # head pairs per core
SB = 128             # s-block size
TC = 512             # attention t-chunk
NTC = T // TC        # 4
NSB = T // SB        # 16
PC = 512             # projection t-chunk (x streaming granularity)
NPC = T // PC        # 8
CCH = D // 128       # 8 contraction chunks

_cache = {}
last_result = {}


def _classify_blocks(mask):
    """Per (s_blk, t_chunk) classification, unioned across batches (SPMD).

    Returns (mode, cls, mixed_list) where cls[s][i] in {0 skip, 1 plain,
    2 mixed} and mixed_list orders the mixed blocks.
    """
    causal = np.triu(np.ones((T, T), dtype=bool), k=1)
    if all(np.array_equal(mask[b], causal) for b in range(B)):
        return "causal", None, None
    cls = np.zeros((NSB, NTC), dtype=np.int64)
    for s in range(NSB):
        for i in range(NTC):
            blk = mask[:, i * TC:(i + 1) * TC, s * SB:(s + 1) * SB]  # [B,t,s]
            if blk.any():
                cls[s, i] = 2 if not blk.all() else 0
            else:
                cls[s, i] = 1
    # a block masked in every batch can still differ per batch -> recheck:
    # skip only if all batches fully masked; mixed if some batch partially
    # or batches disagree (all-masked vs all-valid across batches)
    for s in range(NSB):
        for i in range(NTC):
            blk = mask[:, i * TC:(i + 1) * TC, s * SB:(s + 1) * SB]
            per_b_all = [mask[b, i * TC:(i + 1) * TC, s * SB:(s + 1) * SB].all()
                         for b in range(B)]
            per_b_any = [mask[b, i * TC:(i + 1) * TC, s * SB:(s + 1) * SB].any()
                         for b in range(B)]
            if all(per_b_all):
                cls[s, i] = 0
            elif not any(per_b_any):
                cls[s, i] = 1
            else:
                cls[s, i] = 2
    mixed = [(s, i) for s in range(NSB) for i in range(NTC) if cls[s, i] == 2]
    return "general", cls, mixed


def _build(mode, cls, n_mixed):
    nc = bacc.Bacc("TRN2", target_bir_lowering=False, debug=False,
                   num_devices=NCORE)
    d = {}
    for nm in ("xq", "xk", "xv"):
        d[nm] = nc.dram_tensor(nm, [D, T], BF16, kind="ExternalInput").ap()
    for nm in ("wq", "wk", "wv"):
        d[nm] = nc.dram_tensor(nm, [D, DG], BF16, kind="ExternalInput").ap()
    d["wo"] = nc.dram_tensor("wo", [DG, D], BF16, kind="ExternalInput").ap()
    d["bq"] = nc.dram_tensor("bq", [128, NPAIR], F32, kind="ExternalInput").ap()
    d["bk"] = nc.dram_tensor("bk", [128, NPAIR], F32, kind="ExternalInput").ap()
    d["bv"] = nc.dram_tensor("bv", [1, DG], BF16, kind="ExternalInput").ap()
    d["ones1"] = nc.dram_tensor("ones1", [1, 128], BF16, kind="ExternalInput").ap()
    d["onesv"] = nc.dram_tensor("onesv", [128, H // 2], BF16,
                                kind="ExternalInput").ap()
    if n_mixed:
        d["mmask"] = nc.dram_tensor("mmask", [n_mixed, SB, TC], BF16,
                                    kind="ExternalInput").ap()
    out_d = nc.dram_tensor("out", [T, D], F32, kind="ExternalOutput").ap()

    with tile.TileContext(nc) as tc:
        with (
            tc.tile_pool(name="persist", bufs=1) as pp,
            tc.tile_pool(name="stream", bufs=2) as sp,
            tc.tile_pool(name="small", bufs=2) as mp,
            tc.tile_pool(name="psum", bufs=2, space="PSUM") as psp,
        ):
            HV = HD + 1  # 65: V columns + ones column per head
            # ---- V path first: weights, then projection -----------------
            wv_sb = pp.tile([128, CCH * DG], BF16, tag="wv")
            nc.gpsimd.dma_start(
                out=wv_sb[:].rearrange("p (c e) -> p c e", e=DG),
                in_=d["wv"].rearrange("(c p) e -> p c e", p=128))
            bv_sb = pp.tile([1, DG], BF16, tag="bv")
            ones1_sb = pp.tile([1, 128], BF16, tag="ones1")
            nc.sync.dma_start(out=bv_sb[:], in_=d["bv"][:])
            nc.sync.dma_start(out=ones1_sb[:], in_=d["ones1"][:])
            v_sb = [pp.tile([128, 8 * HV], BF16, tag=f"v{s}", name=f"v{s}") for s in range(NSB)]
            for s in range(NSB):
                ones_cols = v_sb[s][:].rearrange("p (h c) -> p h c", c=HV)[:, :, HD:HV]
                nc.gpsimd.memset(ones_cols, 1.0)

            def emit_v(tau):
                x = sp.tile([128, CCH * PC], BF16, tag="x", bufs=4, name="xv_t")
                nc.gpsimd.dma_start(
                    out=x[:].rearrange("p (c t) -> p c t", t=PC),
                    in_=d["xv"].rearrange("(c p) t -> p c t", p=128)[:, :, ts(tau, PC)])
                for u in range(PC // SB):
                    sigma = tau * (PC // SB) + u
                    ps = psp.tile([128, TC], F32, tag="b512", bufs=2)
                    for c in range(CCH):
                        nc.tensor.matmul(
                            ps[:],
                            x[:, c * PC + u * SB:c * PC + (u + 1) * SB],
                            wv_sb[:, ts(c, DG)],
                            start=(c == 0), stop=False)
                    nc.tensor.matmul(ps[:], ones1_sb[:], bv_sb[:],
                                     start=False, stop=True)
                    vdst = v_sb[sigma][:].rearrange("p (h c) -> p h c", c=HV)[:, :, 0:HD]
                    vsrc = ps[:].rearrange("p (h c) -> p h c", c=HD)
                    nc.vector.tensor_copy(vdst, vsrc)

            # ---- Q/K projections (per-(pair, t-chunk) dest tiles) -------
            w_sb = {}
            for nm in ("wq", "wk"):
                w = pp.tile([128, CCH * DG], BF16, tag=nm, name=nm + "_sb")
                nc.gpsimd.dma_start(
                    out=w[:].rearrange("p (c e) -> p c e", e=DG),
                    in_=d[nm].rearrange("(c p) e -> p c e", p=128))
                w_sb[nm] = w
            bq_sb = pp.tile([128, NPAIR], F32, tag="bq")
            bk_sb = pp.tile([128, NPAIR], F32, tag="bk")
            nc.sync.dma_start(out=bq_sb[:], in_=d["bq"][:])
            nc.sync.dma_start(out=bk_sb[:], in_=d["bk"][:])
            wo_sb = pp.tile([128, NPAIR * D], BF16, tag="wo")
            nc.gpsimd.dma_start(
                out=wo_sb[:].rearrange("p (c e) -> p c e", e=D),
                in_=d["wo"].rearrange("(c p) e -> p c e", p=128))

            qT = [[pp.tile([128, TC], BF16, tag=f"qT{p}_{i}", name=f"qT{p}_{i}")
                   for i in range(NTC)] for p in range(NPAIR)]
            kT = [[pp.tile([128, TC], BF16, tag=f"kT{p}_{i}", name=f"kT{p}_{i}")
                   for i in range(NTC)] for p in range(NPAIR)]
            oT = [[pp.tile([128, TC], BF16, tag=f"oT{p}_{i}", name=f"oT{p}_{i}")
                   for i in range(NTC)] for p in range(NPAIR)]

            qk_x = {}

            def emit_qk(tau, pairs=range(NPAIR)):
                if tau not in qk_x:
                    xq = sp.tile([128, CCH * PC], BF16, tag="x", bufs=4, name="xq_t")
                    xk = sp.tile([128, CCH * PC], BF16, tag="x", bufs=4, name="xk_t")
                    for xx, dd in ((xq, "xq"), (xk, "xk")):
                        nc.gpsimd.dma_start(
                            out=xx[:].rearrange("p (c t) -> p c t", t=PC),
                            in_=d[dd].rearrange("(c p) t -> p c t", p=128)[:, :, ts(tau, PC)])
                    qk_x[tau] = (xq, xk)
                xq, xk = qk_x[tau]
                i = tau
                for p in pairs:
                    for nm, xx, dst, bias in (("q", xq, qT, bq_sb),
                                              ("k", xk, kT, bk_sb)):
                        ps = psp.tile([128, TC], F32, tag="b512", bufs=2)
                        for c in range(CCH):
                            nc.tensor.matmul(
                                ps[:],
                                w_sb["w" + nm][:, c * DG + p * 128:c * DG + (p + 1) * 128],
                                xx[:, ts(c, PC)],
                                start=(c == 0), stop=(c == CCH - 1))
                        nc.vector.tensor_scalar(
                            out=dst[p][i][:], in0=ps[:],
                            scalar1=bias[:, p:p + 1], scalar2=None,
                            op0=mybir.AluOpType.add)

            scale = 1.0 / math.sqrt(HD)

            def build_unit(i, p):
                """Returns (st_fns, pv_fns, epi_fn) for attention unit (i,p)."""
                if mode == "causal":
                    blocks = []
                    for s_blk in range(4 * i + 4):
                        j = s_blk - 4 * i
                        if j < 0:
                            blocks.append((s_blk, i * TC, TC, False))
                        else:
                            s0 = SB * s_blk
                            toff = s0 if j < 3 else s0 - SB
                            blocks.append((s_blk, toff, TC * (i + 1) - toff, True))
                else:
                    blocks = [(s_blk, i * TC, TC, False)
                              for s_blk in range(NSB) if cls[s_blk, i] != 0]
                state = {"p2": {}, "ot": None}

                def make_st(bi):
                    s_blk, toff, n, diag = blocks[bi]

                    def fn():
                        s0 = SB * s_blk
                        sc, lo = s_blk // 4, SB * (s_blk % 4)
                        tl = toff - i * TC
                        st2 = psp.tile([128, 2 * TC], F32, tag="stAB", bufs=2,
                                       name="st2")
                        nc.tensor.matmul(
                            st2[:, 0:n], kT[p][sc][0:HD, lo:lo + SB],
                            qT[p][i][0:HD, tl:tl + n],
                            start=True, stop=True, tile_position=(0, 0))
                        nc.tensor.matmul(
                            st2[:, TC:TC + n], kT[p][sc][HD:128, lo:lo + SB],
                            qT[p][i][HD:128, tl:tl + n],
                            start=True, stop=True, tile_position=(64, 0))
                        p2 = sp.tile([128, 2 * TC], BF16, tag="pAB", bufs=6,
                                     name="p2")
                        if n == TC:
                            nc.scalar.activation(p2[:], st2[:], AF.Exp, scale=scale)
                        else:
                            st3 = st2[:].rearrange("p (b c) -> p b c", b=2)[:, :, 0:n]
                            p3 = p2[:].rearrange("p (b c) -> p b c", b=2)[:, :, 0:n]
                            nc.scalar.activation(p3, st3, AF.Exp, scale=scale)
                        if mode == "causal" and diag:
                            w_ = s0 + SB - toff
                            for off in (0, TC):
                                nc.gpsimd.affine_select(
                                    out=p2[:, off:off + w_], in_=p2[:, off:off + w_],
                                    compare_op=mybir.AluOpType.is_ge,
                                    fill=0.0, base=toff - s0,
                                    pattern=[[1, w_]], channel_multiplier=-1)
                        elif mode == "general" and cls[s_blk, i] == 2:
                            mmt = sp.tile([SB, TC], BF16, tag="mmask", name="mmt")
                            nc.sync.dma_start(out=mmt[:],
                                              in_=d["mmask"][mixed_idx[(s_blk, i)]])
                            for off in (0, TC):
                                nc.vector.tensor_mul(p2[:, off:off + n],
                                                     p2[:, off:off + n], mmt[:, 0:n])
                        state["p2"][bi] = p2
                    return fn

                def make_pv(bi):
                    s_blk, toff, n, diag = blocks[bi]

                    def fn():
                        if state["ot"] is None:
                            state["ot"] = (
                                psp.tile([HV, TC], F32, tag="ot", bufs=2, name="otA"),
                                psp.tile([HV, TC], F32, tag="ot", bufs=2, name="otB"))
                        otA, otB = state["ot"]
                        p2 = state["p2"].pop(bi)
                        tl = toff - i * TC
                        vv = v_sb[s_blk][:].rearrange("p (h c) -> p h c", c=HV)
                        first, last = bi == 0, bi == len(blocks) - 1
                        nc.tensor.matmul(otA[:, tl:tl + n], vv[:, 2 * p, :],
                                         p2[:, 0:n], start=first, stop=last)
                        nc.tensor.matmul(otB[:, tl:tl + n], vv[:, 2 * p + 1, :],
                                         p2[:, TC:TC + n], start=first, stop=last)
                    return fn

                def epi():
                    otA, otB = state["ot"]
                    for hh, ot_ps in ((0, otA), (1, otB)):
                        den = mp.tile([HV, TC], F32, tag="den", name="den")
                        nc.vector.tensor_copy(den[HD:HV, :], ot_ps[HD:HV, :])
                        r1 = mp.tile([1, TC], F32, tag="r1", name="r1")
                        nc.sync.dma_start(out=r1[:], in_=den[HD:HV, :])
                        rbd = mp.tile([HD, TC], F32, tag="rbd", name="rbd")
                        nc.gpsimd.partition_broadcast(rbd[:], r1[:])
                        rb = mp.tile([HD, TC], F32, tag="rb", name="rb")
                        nc.vector.reciprocal_approx_fast(out=rb[:], in_=rbd[:])
                        if hh == 0:
                            nc.vector.tensor_mul(oT[p][i][0:HD, :], ot_ps[0:HD, :],
                                                 rb[:])
                        else:
                            stg = mp.tile([HD, TC], BF16, tag="stg", name="stg")
                            nc.vector.tensor_mul(stg[:], ot_ps[0:HD, :], rb[:])
                            nc.sync.dma_start(out=oT[p][i][HD:128, :], in_=stg[:])

                n = len(blocks)
                return [make_st(b) for b in range(n)], [make_pv(b) for b in range(n)], epi

            def emit_outproj(i):
                for tt in range(4 * i, 4 * i + 4):
                    ob = sp.tile([128, D], F32, tag="ob", bufs=3, name="ob")
                    for e in range(2):
                        ps = psp.tile([128, TC], F32, tag="b512", bufs=2, name="ops")
                        for p in range(NPAIR):
                            nc.tensor.matmul(
                                ps[:], oT[p][i][:, ts(tt - 4 * i, 128)],
                                wo_sb[:, p * D + e * TC:p * D + (e + 1) * TC],
                                start=(p == 0), stop=(p == NPAIR - 1))
                        nc.vector.tensor_copy(ob[:, ts(e, TC)], ps[:])
                    nc.sync.dma_start(out=out_d[ts(tt, 128), :], in_=ob[:])

            # minimal prefix, then pipelined attention emission: PVs lag
            # STs by LAG blocks, and the lag carries across unit boundaries
            # so the next unit's score matmuls precede the previous unit's
            # last PVs in the static per-engine schedule. v/qk projections
            # for chunk i+1 are woven between units of chunk i.
            emit_v(0)
            emit_qk(0)
            LAG = 1
            carry = []  # deferred (pv_fns tail + epilogue) of previous unit

            def emit_unit(st_fns, pv_fns, epi):
                nonlocal_carry = carry[:]
                carry.clear()
                nb = len(st_fns)
                for b in range(min(LAG, nb)):
                    st_fns[b]()
                    if nonlocal_carry:
                        nonlocal_carry.pop(0)()
                for fn in nonlocal_carry:
                    fn()
                for b in range(LAG, nb):
                    st_fns[b]()
                    pv_fns[b - LAG]()
                carry.extend(pv_fns[max(nb - LAG, 0):])
                carry.append(epi)

            for i in range(NTC):
                t2 = i + 1
                if t2 < NPC:
                    pieces = [lambda: emit_v(t2),
                              lambda: emit_qk(t2, (0, 1)),
                              lambda: emit_qk(t2, (2, 3))]
                else:
                    pieces = []
                done = 0
                for p in range(NPAIR):
                    st_fns, pv_fns, epi = build_unit(i, p)
                    emit_unit(st_fns, pv_fns, epi)
                    want = (p + 1) * len(pieces) // NPAIR
                    while done < want:
                        pieces[done]()
                        done += 1
                if i >= 1:
                    emit_outproj(i - 1)
            for fn in carry:
                fn()
            emit_outproj(NTC - 1)

    nc.compile()
    return nc


def kernel(**inputs):
    query = np.asarray(inputs["query"], np.float32)
    key = np.asarray(inputs["key"], np.float32)
    value = np.asarray(inputs["value"], np.float32)
    mask = np.asarray(inputs["mask"], bool)
    Wq, bq = np.asarray(inputs["Wq"], np.float32), np.asarray(inputs["bq"], np.float32)
    Wk, bk = np.asarray(inputs["Wk"], np.float32), np.asarray(inputs["bk"], np.float32)
    Wv, bv = np.asarray(inputs["Wv"], np.float32), np.asarray(inputs["bv"], np.float32)
    Wo, bo = np.asarray(inputs["Wo"], np.float32), np.asarray(inputs["bo"], np.float32)

    mode, cls, mixed = _classify_blocks(mask)
    global mixed_idx
    if mode == "general":
        mixed_idx = {blk: n for n, blk in enumerate(mixed)}
        n_mixed = len(mixed)
    else:
        mixed_idx, n_mixed = {}, 0

    key_sig = (mode, tuple(cls.ravel()) if cls is not None else None)
    if key_sig not in _cache:
        _cache[key_sig] = _build(mode, cls, n_mixed)
    nc = _cache[key_sig]

    in_maps = []
    xT = {}
    for b in range(B):
        xT[("xq", b)] = np.ascontiguousarray(query[b].T).astype(ml_dtypes.bfloat16)
        xT[("xk", b)] = np.ascontiguousarray(key[b].T).astype(ml_dtypes.bfloat16)
        xT[("xv", b)] = np.ascontiguousarray(value[b].T).astype(ml_dtypes.bfloat16)
    for core in range(NCORE):
        b, g = core // 2, core % 2
        sl = slice(g * DG, (g + 1) * DG)
        im = {
            "xq": xT[("xq", b)], "xk": xT[("xk", b)], "xv": xT[("xv", b)],
            "wq": np.ascontiguousarray(Wq[sl, :].T).astype(ml_dtypes.bfloat16),
            "wk": np.ascontiguousarray(Wk[sl, :].T).astype(ml_dtypes.bfloat16),
            "wv": np.ascontiguousarray(Wv[sl, :].T).astype(ml_dtypes.bfloat16),
            "wo": np.ascontiguousarray(Wo[:, sl].T).astype(ml_dtypes.bfloat16),
            "bq": np.ascontiguousarray(bq[sl].reshape(NPAIR, 128).T),
            "bk": np.ascontiguousarray(bk[sl].reshape(NPAIR, 128).T),
            "bv": np.ascontiguousarray(bv[sl])[None, :].astype(ml_dtypes.bfloat16),
            "ones1": np.ones((1, 128), ml_dtypes.bfloat16),
            "onesv": np.ones((128, H // 2), ml_dtypes.bfloat16),
        }
        if n_mixed:
            mm = np.empty((n_mixed, SB, TC), ml_dtypes.bfloat16)
            for n, (s_blk, i) in enumerate(mixed):
                blk = mask[b, i * TC:(i + 1) * TC, s_blk * SB:(s_blk + 1) * SB]
                mm[n] = (~blk.T).astype(np.float32)
            im["mmask"] = mm
        in_maps.append(im)

    r = run_bass_kernel_spmd(nc, in_maps, core_ids=list(range(NCORE)))
    last_result["exec_time_ns"] = r.exec_time_ns
    last_result["profile_json"] = getattr(r, "profile_json", None)
    last_result["instructions_and_trace"] = getattr(r, "instructions_and_trace", None)
    out = np.empty((B, T, D), np.float32)
    for b in range(B):
        out[b] = r.results[2 * b]["out"] + r.results[2 * b + 1]["out"]
    out += bo[None, None, :]
    return out



# revision 13
# speedup vs baseline: 1.0233x; 1.0233x over previous
"""Cached multi-head attention on 8 TRN2 NeuronCores.

Sharding: core c = 2*b + g handles batch b (of 4) and head-group g (of 2,
8 heads each) -- data parallel on batch x tensor parallel on heads.
Column-parallel Wq/Wk/Wv, row-parallel Wo; the Wo all-reduce (sum of the
two head-group partials per batch) is done on host during the unshard,
along with the bo bias add.

Device schedule (per core): the exp on the Scalar engine (~1.1us per
128x1024 score block) is the pacing resource during attention, while the
PE has ~450ns of slack per block.  So attention blocks are emitted as a
flat stream (ST_k issued, PV_{k-LAG} trailing) and all projection /
out-projection matmuls are queued as small "fill" closures consumed one
per block slot, keeping both PE and Scalar continuously busy.  Startup
DMAs are spread across four engine queues so the first v-projection
matmul can start ~4us in.

Causal masks get a fast path: blocks above the diagonal are skipped,
diagonal blocks use shortened matmuls + gpsimd affine_select zeroing.
Arbitrary masks fall back to per-block skip/plain/mixed classification
with host-shipped multiplicative mask tiles.
"""

import math
from collections import deque

import ml_dtypes
import numpy as np

import concourse.bass as bass
import concourse.mybir as mybir
import concourse.tile as tile
from concourse import bacc
from concourse.bass_utils import run_bass_kernel_spmd

F32 = mybir.dt.float32
BF16 = mybir.dt.bfloat16
AF = mybir.ActivationFunctionType
ts = bass.ts

B, T, D, H = 4, 2048, 1024, 16
HD = D // H          # 64
NCORE = 8
DG = D // 2          # 512 dims per core (8 heads)
NPAIR = 4            # head pairs per core
SB = 128             # s-block size
TC = 512             # attention t-chunk
NTC = T // TC        # 4
NSB = T // SB        # 16
PC = 512             # projection t-chunk (x streaming granularity)
CCH = D // 128       # 8 contraction chunks

LAG = 2              # PV trails ST by this many blocks

_cache = {}
last_result = {}


def _classify_blocks(mask):
    """Per (s_blk, t_chunk) classification, unioned across batches (SPMD).

    Returns (mode, cls, mixed_list) where cls[s][i] in {0 skip, 1 plain,
    2 mixed} and mixed_list orders the mixed blocks.
    """
    causal = np.triu(np.ones((T, T), dtype=bool), k=1)
    if all(np.array_equal(mask[b], causal) for b in range(B)):
        return "causal", None, None
    cls = np.zeros((NSB, NTC), dtype=np.int64)
    for s in range(NSB):
        for i in range(NTC):
            per_b_all = [mask[b, i * TC:(i + 1) * TC, s * SB:(s + 1) * SB].all()
                         for b in range(B)]
            per_b_any = [mask[b, i * TC:(i + 1) * TC, s * SB:(s + 1) * SB].any()
                         for b in range(B)]
            if all(per_b_all):
                cls[s, i] = 0
            elif not any(per_b_any):
                cls[s, i] = 1
            else:
                cls[s, i] = 2
    mixed = [(s, i) for s in range(NSB) for i in range(NTC) if cls[s, i] == 2]
    return "general", cls, mixed


def _build(mode, cls, n_mixed):
    nc = bacc.Bacc("TRN2", target_bir_lowering=False, debug=False,
                   num_devices=NCORE)
    d = {}
    for nm in ("xq", "xk", "xv"):
        d[nm] = nc.dram_tensor(nm, [D, T], BF16, kind="ExternalInput").ap()
    for nm in ("wq", "wk", "wv"):
        d[nm] = nc.dram_tensor(nm, [D, DG], BF16, kind="ExternalInput").ap()
    d["wo"] = nc.dram_tensor("wo", [DG, D], BF16, kind="ExternalInput").ap()
    d["bq"] = nc.dram_tensor("bq", [128, NPAIR], F32, kind="ExternalInput").ap()
    d["bk"] = nc.dram_tensor("bk", [128, NPAIR], F32, kind="ExternalInput").ap()
    d["bv"] = nc.dram_tensor("bv", [1, DG], BF16, kind="ExternalInput").ap()
    d["ones1"] = nc.dram_tensor("ones1", [1, 128], BF16, kind="ExternalInput").ap()
    d["onesv"] = nc.dram_tensor("onesv", [128, H // 2], BF16,
                                kind="ExternalInput").ap()
    if n_mixed:
        d["mmask"] = nc.dram_tensor("mmask", [n_mixed, SB, TC], BF16,
                                    kind="ExternalInput").ap()
    out_d = nc.dram_tensor("out", [T, D], F32, kind="ExternalOutput").ap()

    with tile.TileContext(nc) as tc:
        with (
            tc.tile_pool(name="persist", bufs=1) as pp,
            tc.tile_pool(name="stream", bufs=2) as sp,
            tc.tile_pool(name="small", bufs=2) as mp,
            tc.tile_pool(name="psum", bufs=2, space="PSUM") as psp,
        ):
            HV = HD + 1  # 65: V columns + ones column per head

            # ---- persistent tiles --------------------------------------
            wv_sb = pp.tile([128, CCH * DG], BF16, tag="wv")
            wq_sb = pp.tile([128, CCH * DG], BF16, tag="wq")
            wk_sb = pp.tile([128, CCH * DG], BF16, tag="wk")
            wo_sb = pp.tile([128, NPAIR * D], BF16, tag="wo")
            bq_sb = pp.tile([128, NPAIR], F32, tag="bq")
            bk_sb = pp.tile([128, NPAIR], F32, tag="bk")
            bv_sb = pp.tile([1, DG], BF16, tag="bv")
            ones1_sb = pp.tile([1, 128], BF16, tag="ones1")
            v_sb = [pp.tile([128, 8 * HV], BF16, tag=f"v{s}", name=f"v{s}")
                    for s in range(NSB)]
            w_sb = {"wq": wq_sb, "wk": wk_sb}

            def wload(eng, w, nm):
                eng.dma_start(
                    out=w[:].rearrange("p (c e) -> p c e", e=DG),
                    in_=d[nm].rearrange("(c p) e -> p c e", p=128))

            # ---- startup DMAs spread across queues ---------------------
            # scalar (Act) queue is idle until the first exp: smalls, wv, wk
            nc.scalar.dma_start(out=bv_sb[:], in_=d["bv"][:])
            nc.scalar.dma_start(out=ones1_sb[:], in_=d["ones1"][:])
            nc.scalar.dma_start(out=bq_sb[:], in_=d["bq"][:])
            nc.scalar.dma_start(out=bk_sb[:], in_=d["bk"][:])
            wload(nc.scalar, wv_sb, "wv")
            wload(nc.scalar, wk_sb, "wk")
            # gpsimd queue: wq, then the v ones-columns memsets
            wload(nc.gpsimd, wq_sb, "wq")
            for s in range(NSB):
                ones_cols = v_sb[s][:].rearrange("p (h c) -> p h c", c=HV)[:, :, HD:HV]
                nc.gpsimd.memset(ones_cols, 1.0)

            # x chunk streaming (issued across queues; on-demand guard)
            x_tiles = {}
            _xq_rr = [0]

            def load_x(kind, tau, eng=None):
                if (kind, tau) in x_tiles:
                    return
                if eng is None:
                    eng = (nc.sync, nc.gpsimd)[_xq_rr[0] % 2]
                    _xq_rr[0] += 1
                xx = sp.tile([128, CCH * PC], BF16, tag=kind, bufs=2,
                             name=f"{kind}{tau}")
                eng.dma_start(
                    out=xx[:].rearrange("p (c t) -> p c t", t=PC),
                    in_=d[kind].rearrange("(c p) t -> p c t", p=128)[:, :, ts(tau, PC)])
                x_tiles[(kind, tau)] = xx

            load_x("xv", 0, nc.sync)
            load_x("xq", 0, nc.sync)
            load_x("xk", 0, nc.gpsimd)
            wload(nc.gpsimd, wo_sb, "wo")  # needed late (out-projection)

            qT = [[pp.tile([128, TC], BF16, tag=f"qT{p}_{i}", name=f"qT{p}_{i}")
                   for i in range(NTC)] for p in range(NPAIR)]
            kT = [[pp.tile([128, TC], BF16, tag=f"kT{p}_{i}", name=f"kT{p}_{i}")
                   for i in range(NTC)] for p in range(NPAIR)]
            oT = [[pp.tile([128, TC], BF16, tag=f"oT{p}_{i}", name=f"oT{p}_{i}")
                   for i in range(NTC)] for p in range(NPAIR)]

            # ---- fill closures (projections / out-projection) ----------
            # each closure is ~4-5 matmuls; consumed one per block slot.
            def v_fills(sigma):
                tau, u = sigma // 4, sigma % 4
                st_ = {}

                def a():
                    load_x("xv", tau)
                    x = x_tiles[("xv", tau)]
                    ps = psp.tile([128, TC], F32, tag="b512", bufs=2)
                    for c in range(4):
                        nc.tensor.matmul(
                            ps[:], x[:, c * PC + u * SB:c * PC + (u + 1) * SB],
                            wv_sb[:, ts(c, DG)], start=(c == 0), stop=False)
                    st_["ps"] = ps

                def b():
                    x = x_tiles[("xv", tau)]
                    ps = st_.pop("ps")
                    for c in range(4, CCH):
                        nc.tensor.matmul(
                            ps[:], x[:, c * PC + u * SB:c * PC + (u + 1) * SB],
                            wv_sb[:, ts(c, DG)], start=False, stop=False)
                    nc.tensor.matmul(ps[:], ones1_sb[:], bv_sb[:],
                                     start=False, stop=True)
                    vdst = v_sb[sigma][:].rearrange("p (h c) -> p h c", c=HV)[:, :, 0:HD]
                    nc.vector.tensor_copy(vdst, ps[:].rearrange("p (h c) -> p h c", c=HD))

                return [a, b]

            def qk_fills(nm, p, i):
                # nm in ("q", "k"); computes dst[p][i] = W x + bias
                st_ = {}
                dst = qT if nm == "q" else kT
                bias = bq_sb if nm == "q" else bk_sb
                xkind = "xq" if nm == "q" else "xk"

                def a():
                    load_x(xkind, i)
                    xx = x_tiles[(xkind, i)]
                    ps = psp.tile([128, TC], F32, tag="b512", bufs=2)
                    for c in range(4):
                        nc.tensor.matmul(
                            ps[:],
                            w_sb["w" + nm][:, c * DG + p * 128:c * DG + (p + 1) * 128],
                            xx[:, ts(c, PC)], start=(c == 0), stop=False)
                    st_["ps"] = ps

                def b():
                    xx = x_tiles[(xkind, i)]
                    ps = st_.pop("ps")
                    for c in range(4, CCH):
                        nc.tensor.matmul(
                            ps[:],
                            w_sb["w" + nm][:, c * DG + p * 128:c * DG + (p + 1) * 128],
                            xx[:, ts(c, PC)], start=False, stop=(c == CCH - 1))
                    nc.vector.tensor_scalar(
                        out=dst[p][i][:], in0=ps[:],
                        scalar1=bias[:, p:p + 1], scalar2=None,
                        op0=mybir.AluOpType.add)

                return [a, b]

            def outproj_fills(i, tt):
                # out[tt*128:(tt+1)*128, :] = sum_p oT[p][i] @ wo
                st_ = {}

                def a():
                    ps = psp.tile([128, TC], F32, tag="b512", bufs=2, name="ops")
                    for p in range(NPAIR):
                        nc.tensor.matmul(
                            ps[:], oT[p][i][:, ts(tt - 4 * i, 128)],
                            wo_sb[:, p * D + 0 * TC:p * D + 1 * TC],
                            start=(p == 0), stop=(p == NPAIR - 1))
                    st_["ps0"] = ps

                def b():
                    ob = sp.tile([128, D], F32, tag="ob", bufs=2, name="ob")
                    st_["ob"] = ob
                    nc.vector.tensor_copy(ob[:, ts(0, TC)], st_.pop("ps0")[:])
                    ps = psp.tile([128, TC], F32, tag="b512", bufs=2, name="ops")
                    for p in range(NPAIR):
                        nc.tensor.matmul(
                            ps[:], oT[p][i][:, ts(tt - 4 * i, 128)],
                            wo_sb[:, p * D + 1 * TC:p * D + 2 * TC],
                            start=(p == 0), stop=(p == NPAIR - 1))
                    st_["ps1"] = ps

                def c():
                    ob = st_.pop("ob")
                    nc.vector.tensor_copy(ob[:, ts(1, TC)], st_.pop("ps1")[:])
                    nc.sync.dma_start(out=out_d[ts(tt, 128), :], in_=ob[:])

                return [a, b, c]

            # fill queue: (marker, fn).  All fills with marker <= u must be
            # emitted before unit u's first ST.
            fills = deque()

            def drain_until(u):
                while fills and fills[0][0] <= u:
                    fills.popleft()[1]()

            def pump(n):
                for _ in range(n):
                    if not fills:
                        return
                    fills.popleft()[1]()

            # deadline one unit early so the DVE drain lands before use
            for um in range(1, NTC * NPAIR):
                i, p = um // 4, um % 4
                if p == 0:
                    for sg in range(4 * i, 4 * i + 4):
                        for fn in v_fills(sg):
                            fills.append((um - 1, fn))
                for nm in ("q", "k"):
                    for fn in qk_fills(nm, p, i):
                        fills.append((um - 1, fn))

            # ---- attention unit construction ---------------------------
            scale = 1.0 / math.sqrt(HD)

            def build_unit(i, p):
                """Returns (st_fns, pv_fns, epi_fn) for attention unit (i,p)."""
                if mode == "causal":
                    blocks = []
                    for s_blk in range(4 * i + 4):
                        j = s_blk - 4 * i
                        if j < 0:
                            blocks.append((s_blk, i * TC, TC, False))
                        else:
                            s0 = SB * s_blk
                            toff = s0 if j < 3 else s0 - SB
                            blocks.append((s_blk, toff, TC * (i + 1) - toff, True))
                else:
                    blocks = [(s_blk, i * TC, TC, False)
                              for s_blk in range(NSB) if cls[s_blk, i] != 0]
                state = {"p2": {}, "ot": None}

                def make_st(bi):
                    s_blk, toff, n, diag = blocks[bi]

                    def fn():
                        s0 = SB * s_blk
                        sc, lo = s_blk // 4, SB * (s_blk % 4)
                        tl = toff - i * TC
                        st2 = psp.tile([128, 2 * TC], F32, tag="stAB", bufs=2,
                                       name="st2")
                        nc.tensor.matmul(
                            st2[:, 0:n], kT[p][sc][0:HD, lo:lo + SB],
                            qT[p][i][0:HD, tl:tl + n],
                            start=True, stop=True, tile_position=(0, 0))
                        nc.tensor.matmul(
                            st2[:, TC:TC + n], kT[p][sc][HD:128, lo:lo + SB],
                            qT[p][i][HD:128, tl:tl + n],
                            start=True, stop=True, tile_position=(64, 0))
                        p2 = sp.tile([128, 2 * TC], BF16, tag="pAB", bufs=8,
                                     name="p2")
                        if n == TC:
                            nc.scalar.activation(p2[:], st2[:], AF.Exp, scale=scale)
                        else:
                            st3 = st2[:].rearrange("p (b c) -> p b c", b=2)[:, :, 0:n]
                            p3 = p2[:].rearrange("p (b c) -> p b c", b=2)[:, :, 0:n]
                            nc.scalar.activation(p3, st3, AF.Exp, scale=scale)
                        if mode == "causal" and diag:
                            w_ = s0 + SB - toff
                            for off in (0, TC):
                                nc.gpsimd.affine_select(
                                    out=p2[:, off:off + w_], in_=p2[:, off:off + w_],
                                    compare_op=mybir.AluOpType.is_ge,
                                    fill=0.0, base=toff - s0,
                                    pattern=[[1, w_]], channel_multiplier=-1)
                        elif mode == "general" and cls[s_blk, i] == 2:
                            mmt = sp.tile([SB, TC], BF16, tag="mmask", name="mmt")
                            nc.sync.dma_start(out=mmt[:],
                                              in_=d["mmask"][mixed_idx[(s_blk, i)]])
                            for off in (0, TC):
                                nc.vector.tensor_mul(p2[:, off:off + n],
                                                     p2[:, off:off + n], mmt[:, 0:n])
                        state["p2"][bi] = p2
                    return fn

                def make_pv(bi):
                    s_blk, toff, n, diag = blocks[bi]

                    def fn():
                        if state["ot"] is None:
                            state["ot"] = (
                                psp.tile([HV, TC], F32, tag="ot", bufs=2, name="otA"),
                                psp.tile([HV, TC], F32, tag="ot", bufs=2, name="otB"))
                        otA, otB = state["ot"]
                        p2 = state["p2"].pop(bi)
                        tl = toff - i * TC
                        vv = v_sb[s_blk][:].rearrange("p (h c) -> p h c", c=HV)
                        first, last = bi == 0, bi == len(blocks) - 1
                        nc.tensor.matmul(otA[:, tl:tl + n], vv[:, 2 * p, :],
                                         p2[:, 0:n], start=first, stop=last)
                        nc.tensor.matmul(otB[:, tl:tl + n], vv[:, 2 * p + 1, :],
                                         p2[:, TC:TC + n], start=first, stop=last)
                    return fn

                def epi():
                    otA, otB = state["ot"]
                    for hh, ot_ps in ((0, otA), (1, otB)):
                        # copy PSUM out up-front so the bank frees quickly
                        cp = mp.tile([HV, TC], F32, tag="ocp", bufs=4, name="ocp")
                        nc.vector.tensor_copy(cp[:], ot_ps[:])
                        r1 = mp.tile([1, TC], F32, tag="r1", name="r1")
                        nc.sync.dma_start(out=r1[:], in_=cp[HD:HV, :])
                        rbd = mp.tile([HD, TC], F32, tag="rbd", name="rbd")
                        nc.gpsimd.partition_broadcast(rbd[:], r1[:])
                        rb = mp.tile([HD, TC], F32, tag="rb", name="rb")
                        nc.vector.reciprocal_approx_fast(out=rb[:], in_=rbd[:])
                        if hh == 0:
                            nc.vector.tensor_mul(oT[p][i][0:HD, :], cp[0:HD, :],
                                                 rb[:])
                        else:
                            stg = mp.tile([HD, TC], BF16, tag="stg", name="stg")
                            nc.vector.tensor_mul(stg[:], cp[0:HD, :], rb[:])
                            nc.gpsimd.dma_start(out=oT[p][i][HD:128, :], in_=stg[:])

                n = len(blocks)
                return ([make_st(b) for b in range(n)],
                        [make_pv(b) for b in range(n)], epi)

            # ---- prologue: chunk-0 s-blocks + pair-0 q/k ---------------
            for sg in range(4):
                for fn in v_fills(sg):
                    fn()
            for nm in ("q", "k"):
                for fn in qk_fills(nm, 0, 0):
                    fn()
            # chunk-1 x prefetch (issues only; transfers overlap chunk 0)
            for kind in ("xv", "xq", "xk"):
                load_x(kind, 1)

            # ---- flat block stream with LAG ----------------------------
            units = [(i, p) for i in range(NTC) for p in range(NPAIR)]
            stream = []
            epis = {}
            for u, (i, p) in enumerate(units):
                st_fns, pv_fns, epi = build_unit(i, p)
                epis[u] = epi
                nb = len(st_fns)
                for b in range(nb):
                    stream.append((st_fns[b], pv_fns[b], u, b == nb - 1))

            def after_chunk(i):
                # runs shortly after chunk i's last epilogue is emitted
                if i + 2 < NTC:
                    for kind in ("xv", "xq", "xk"):
                        load_x(kind, i + 2)
                um = min(4 * (i + 1) + 2, NTC * NPAIR - 1)
                for tt in range(4 * i, 4 * i + 4):
                    for fn in outproj_fills(i, tt):
                        fills.append((um, fn))

            nblocks = len(stream)
            for k in range(nblocks + LAG):
                if k < nblocks:
                    stf, _, u, _ = stream[k]
                    drain_until(u)
                    stf()
                j = k - LAG
                if j >= 0:
                    _, pvf, u, last = stream[j]
                    pvf()
                    if last:
                        epis[u]()
                        i, p = units[u]
                        if p == NPAIR - 1:
                            after_chunk(i)
                pump(1)
            while fills:
                fills.popleft()[1]()

    nc.compile()
    return nc


def kernel(**inputs):
    query = np.asarray(inputs["query"], np.float32)
    key = np.asarray(inputs["key"], np.float32)
    value = np.asarray(inputs["value"], np.float32)
    mask = np.asarray(inputs["mask"], bool)
    Wq, bq = np.asarray(inputs["Wq"], np.float32), np.asarray(inputs["bq"], np.float32)
    Wk, bk = np.asarray(inputs["Wk"], np.float32), np.asarray(inputs["bk"], np.float32)
    Wv, bv = np.asarray(inputs["Wv"], np.float32), np.asarray(inputs["bv"], np.float32)
    Wo, bo = np.asarray(inputs["Wo"], np.float32), np.asarray(inputs["bo"], np.float32)

    mode, cls, mixed = _classify_blocks(mask)
    global mixed_idx
    if mode == "general":
        mixed_idx = {blk: n for n, blk in enumerate(mixed)}
        n_mixed = len(mixed)
    else:
        mixed_idx, n_mixed = {}, 0

    key_sig = (mode, tuple(cls.ravel()) if cls is not None else None)
    if key_sig not in _cache:
        _cache[key_sig] = _build(mode, cls, n_mixed)
    nc = _cache[key_sig]

    in_maps = []
    xT = {}
    for b in range(B):
        xT[("xq", b)] = np.ascontiguousarray(query[b].T).astype(ml_dtypes.bfloat16)
        xT[("xk", b)] = np.ascontiguousarray(key[b].T).astype(ml_dtypes.bfloat16)
        xT[("xv", b)] = np.ascontiguousarray(value[b].T).astype(ml_dtypes.bfloat16)
    for core in range(NCORE):
        b, g = core // 2, core % 2
        sl = slice(g * DG, (g + 1) * DG)
        im = {
            "xq": xT[("xq", b)], "xk": xT[("xk", b)], "xv": xT[("xv", b)],
            "wq": np.ascontiguousarray(Wq[sl, :].T).astype(ml_dtypes.bfloat16),
            "wk": np.ascontiguousarray(Wk[sl, :].T).astype(ml_dtypes.bfloat16),
            "wv": np.ascontiguousarray(Wv[sl, :].T).astype(ml_dtypes.bfloat16),
            "wo": np.ascontiguousarray(Wo[:, sl].T).astype(ml_dtypes.bfloat16),
            "bq": np.ascontiguousarray(bq[sl].reshape(NPAIR, 128).T),
            "bk": np.ascontiguousarray(bk[sl].reshape(NPAIR, 128).T),
            "bv": np.ascontiguousarray(bv[sl])[None, :].astype(ml_dtypes.bfloat16),
            "ones1": np.ones((1, 128), ml_dtypes.bfloat16),
            "onesv": np.ones((128, H // 2), ml_dtypes.bfloat16),
        }
        if n_mixed:
            mm = np.empty((n_mixed, SB, TC), ml_dtypes.bfloat16)
            for n, (s_blk, i) in enumerate(mixed):
                blk = mask[b, i * TC:(i + 1) * TC, s_blk * SB:(s_blk + 1) * SB]
                mm[n] = (~blk.T).astype(np.float32)
            im["mmask"] = mm
        in_maps.append(im)

    r = run_bass_kernel_spmd(nc, in_maps, core_ids=list(range(NCORE)))
    last_result["exec_time_ns"] = r.exec_time_ns
    last_result["profile_json"] = getattr(r, "profile_json", None)
    last_result["instructions_and_trace"] = getattr(r, "instructions_and_trace", None)
    out = np.empty((B, T, D), np.float32)
    for b in range(B):
        out[b] = r.results[2 * b]["out"] + r.results[2 * b + 1]["out"]
    out += bo[None, None, :]
    return out


# revision 17
# speedup vs baseline: 1.0315x; 1.0080x over previous
"""Cached multi-head attention on 8 TRN2 NeuronCores.

Sharding: core c = 2*b + g handles batch b (of 4) and head-group g (of 2,
8 heads each) -- data parallel on batch x tensor parallel on heads.
Column-parallel Wq/Wk/Wv, row-parallel Wo; the Wo all-reduce (sum of the
two head-group partials per batch) is done on host during the unshard,
along with the bo bias add.

Device schedule (per core): the exp on the Scalar engine (~1.1us per
128x1024 score block) and the PE (~218us of matmul streaming) are kept
continuously busy by emitting attention blocks as a flat stream (ST_k
issued, PV_{k-LAG} trailing) with projection / out-projection matmuls
queued as small fill closures consumed one per block slot.  All weights
and activations are pre-swizzled on the host into SBUF layout so every
DMA is a contiguous 128-descriptor transfer, and startup DMAs are
ordered most-critical-first across the three DGE queues.

Causal masks get a fast path: blocks above the diagonal are skipped,
diagonal blocks use shortened matmuls + gpsimd affine_select zeroing.
Arbitrary masks fall back to per-block skip/plain/mixed classification
with host-shipped multiplicative mask tiles.
"""

import math
from collections import deque

import ml_dtypes
import numpy as np

import concourse.bass as bass
import concourse.mybir as mybir
import concourse.tile as tile
from concourse import bacc
from concourse.bass_utils import run_bass_kernel_spmd

F32 = mybir.dt.float32
BF16 = mybir.dt.bfloat16
AF = mybir.ActivationFunctionType
ts = bass.ts

B, T, D, H = 4, 2048, 1024, 16
HD = D // H          # 64
NCORE = 8
DG = D // 2          # 512 dims per core (8 heads)
NPAIR = 4            # head pairs per core
SB = 128             # s-block size
TC = 512             # attention t-chunk
NTC = T // TC        # 4
NSB = T // SB        # 16
PC = 512             # projection t-chunk (x streaming granularity)
CCH = D // 128       # 8 contraction chunks

LAG = 2              # PV trails ST by this many blocks

_cache = {}
last_result = {}


def _classify_blocks(mask):
    """Per (s_blk, t_chunk) classification, unioned across batches (SPMD)."""
    causal = np.triu(np.ones((T, T), dtype=bool), k=1)
    if all(np.array_equal(mask[b], causal) for b in range(B)):
        return "causal", None, None
    cls = np.zeros((NSB, NTC), dtype=np.int64)
    for s in range(NSB):
        for i in range(NTC):
            per_b_all = [mask[b, i * TC:(i + 1) * TC, s * SB:(s + 1) * SB].all()
                         for b in range(B)]
            per_b_any = [mask[b, i * TC:(i + 1) * TC, s * SB:(s + 1) * SB].any()
                         for b in range(B)]
            if all(per_b_all):
                cls[s, i] = 0
            elif not any(per_b_any):
                cls[s, i] = 1
            else:
                cls[s, i] = 2
    mixed = [(s, i) for s in range(NSB) for i in range(NTC) if cls[s, i] == 2]
    return "general", cls, mixed


def _build(mode, cls, n_mixed):
    nc = bacc.Bacc("TRN2", target_bir_lowering=False, debug=False,
                   num_devices=NCORE)
    d = {}
    # host pre-swizzled layouts: every DMA is contiguous per partition
    for nm in ("xq", "xk", "xv"):
        d[nm] = nc.dram_tensor(nm, [128, NTC, CCH, PC], BF16,
                               kind="ExternalInput").ap()
    for nm in ("wq", "wk"):
        d[nm] = nc.dram_tensor(nm, [128, NPAIR, CCH * 128], BF16,
                               kind="ExternalInput").ap()
    d["wv"] = nc.dram_tensor("wv", [128, CCH * DG], BF16, kind="ExternalInput").ap()
    d["wo"] = nc.dram_tensor("wo", [128, NPAIR * D], BF16, kind="ExternalInput").ap()
    d["bq"] = nc.dram_tensor("bq", [128, NPAIR], F32, kind="ExternalInput").ap()
    d["bk"] = nc.dram_tensor("bk", [128, NPAIR], F32, kind="ExternalInput").ap()
    d["bv"] = nc.dram_tensor("bv", [1, DG], BF16, kind="ExternalInput").ap()
    d["ones1"] = nc.dram_tensor("ones1", [1, 128], BF16, kind="ExternalInput").ap()
    if n_mixed:
        d["mmask"] = nc.dram_tensor("mmask", [n_mixed, SB, TC], BF16,
                                    kind="ExternalInput").ap()
    out_d = nc.dram_tensor("out", [T, D], F32, kind="ExternalOutput").ap()

    with tile.TileContext(nc) as tc:
        with (
            tc.tile_pool(name="persist", bufs=1) as pp,
            tc.tile_pool(name="stream", bufs=2) as sp,
            tc.tile_pool(name="small", bufs=2) as mp,
            tc.tile_pool(name="psum", bufs=2, space="PSUM") as psp,
        ):
            HV = HD + 1  # 65: V columns + ones column per head

            # ---- persistent tiles --------------------------------------
            wv_sb = pp.tile([128, CCH * DG], BF16, tag="wv")
            wq_sb = pp.tile([128, NPAIR * CCH * 128], BF16, tag="wq")
            wk_sb = pp.tile([128, NPAIR * CCH * 128], BF16, tag="wk")
            wo_sb = pp.tile([128, NPAIR * D], BF16, tag="wo")
            bq_sb = pp.tile([128, NPAIR], F32, tag="bq")
            bk_sb = pp.tile([128, NPAIR], F32, tag="bk")
            bv_sb = pp.tile([1, DG], BF16, tag="bv")
            ones1_sb = pp.tile([1, 128], BF16, tag="ones1")
            v_sb = [pp.tile([128, 8 * HV], BF16, tag=f"v{s}", name=f"v{s}")
                    for s in range(NSB)]
            w_sb = {"wq": wq_sb, "wk": wk_sb}

            # ---- startup DMAs, most-critical-first ---------------------
            # scalar (Act) queue is idle until the first exp
            nc.scalar.dma_start(out=wv_sb[:], in_=d["wv"][:])
            PW = CCH * 128  # columns per pair in wq/wk sbuf layout
            nc.scalar.dma_start(out=wq_sb[:, 0:PW], in_=d["wq"][:, 0])
            nc.scalar.dma_start(out=wk_sb[:, 0:PW], in_=d["wk"][:, 0])
            nc.scalar.dma_start(out=bv_sb[:], in_=d["bv"][:])
            nc.scalar.dma_start(out=ones1_sb[:], in_=d["ones1"][:])
            nc.scalar.dma_start(out=bq_sb[:], in_=d["bq"][:])
            nc.scalar.dma_start(out=bk_sb[:], in_=d["bk"][:])
            nc.scalar.dma_start(out=wq_sb[:, PW:], in_=d["wq"].rearrange(
                "p q e -> p (q e)")[:, PW:])
            nc.scalar.dma_start(out=wk_sb[:, PW:], in_=d["wk"].rearrange(
                "p q e -> p (q e)")[:, PW:])
            # gpsimd queue: the v ones-columns memsets
            for s in range(NSB):
                ones_cols = v_sb[s][:].rearrange("p (h c) -> p h c", c=HV)[:, :, HD:HV]
                nc.gpsimd.memset(ones_cols, 1.0)

            # x chunk streaming (sync queue first, then round-robin)
            x_tiles = {}
            _xq_rr = [0]

            def load_x(kind, tau, eng=None):
                if (kind, tau) in x_tiles:
                    return
                if eng is None:
                    eng = (nc.sync, nc.gpsimd)[_xq_rr[0] % 2]
                    _xq_rr[0] += 1
                xx = sp.tile([128, CCH * PC], BF16, tag=kind, bufs=2,
                             name=f"{kind}{tau}")
                eng.dma_start(
                    out=xx[:].rearrange("p (c t) -> p c t", t=PC),
                    in_=d[kind][:, tau])
                x_tiles[(kind, tau)] = xx

            load_x("xv", 0, nc.sync)
            load_x("xq", 0, nc.sync)
            load_x("xk", 0, nc.sync)

            qT = [[pp.tile([128, TC], BF16, tag=f"qT{p}_{i}", name=f"qT{p}_{i}")
                   for i in range(NTC)] for p in range(NPAIR)]
            kT = [[pp.tile([128, TC], BF16, tag=f"kT{p}_{i}", name=f"kT{p}_{i}")
                   for i in range(NTC)] for p in range(NPAIR)]
            oT = [[pp.tile([128, TC], BF16, tag=f"oT{p}_{i}", name=f"oT{p}_{i}")
                   for i in range(NTC)] for p in range(NPAIR)]

            # ---- fill closures -----------------------------------------
            def v_fills(sigma):
                tau, u = sigma // 4, sigma % 4
                st_ = {}

                def a():
                    load_x("xv", tau)
                    x = x_tiles[("xv", tau)]
                    ps = psp.tile([128, TC], F32, tag="b512", bufs=2)
                    for c in range(4):
                        nc.tensor.matmul(
                            ps[:], x[:, c * PC + u * SB:c * PC + (u + 1) * SB],
                            wv_sb[:, ts(c, DG)], start=(c == 0), stop=False)
                    st_["ps"] = ps

                def b():
                    x = x_tiles[("xv", tau)]
                    ps = st_.pop("ps")
                    for c in range(4, CCH):
                        nc.tensor.matmul(
                            ps[:], x[:, c * PC + u * SB:c * PC + (u + 1) * SB],
                            wv_sb[:, ts(c, DG)], start=False, stop=False)
                    nc.tensor.matmul(ps[:], ones1_sb[:], bv_sb[:],
                                     start=False, stop=True)
                    vdst = v_sb[sigma][:].rearrange("p (h c) -> p h c", c=HV)[:, :, 0:HD]
                    nc.vector.tensor_copy(vdst, ps[:].rearrange("p (h c) -> p h c", c=HD))

                return [a, b]

            def qk_fills(nm, p, i):
                st_ = {}
                dst = qT if nm == "q" else kT
                bias = bq_sb if nm == "q" else bk_sb
                xkind = "xq" if nm == "q" else "xk"
                w = w_sb["w" + nm]

                def a():
                    load_x(xkind, i)
                    xx = x_tiles[(xkind, i)]
                    ps = psp.tile([128, TC], F32, tag="b512", bufs=2)
                    for c in range(4):
                        nc.tensor.matmul(
                            ps[:], w[:, (p * CCH + c) * 128:(p * CCH + c + 1) * 128],
                            xx[:, ts(c, PC)], start=(c == 0), stop=False)
                    st_["ps"] = ps

                def b():
                    xx = x_tiles[(xkind, i)]
                    ps = st_.pop("ps")
                    for c in range(4, CCH):
                        nc.tensor.matmul(
                            ps[:], w[:, (p * CCH + c) * 128:(p * CCH + c + 1) * 128],
                            xx[:, ts(c, PC)], start=False, stop=(c == CCH - 1))
                    nc.vector.tensor_scalar(
                        out=dst[p][i][:], in0=ps[:],
                        scalar1=bias[:, p:p + 1], scalar2=None,
                        op0=mybir.AluOpType.add)

                return [a, b]

            def outproj_fills(i, tt):
                # chunks 0..2: full 4-pair accumulation per (tt, e)
                st_ = {}

                def a():
                    ps = psp.tile([128, TC], F32, tag="b512", bufs=2, name="ops")
                    for p in range(NPAIR):
                        nc.tensor.matmul(
                            ps[:], oT[p][i][:, ts(tt - 4 * i, 128)],
                            wo_sb[:, p * D + 0 * TC:p * D + 1 * TC],
                            start=(p == 0), stop=(p == NPAIR - 1))
                    st_["ps0"] = ps

                def b():
                    ob = mp.tile([128, D], F32, tag="ob", bufs=4, name="ob")
                    st_["ob"] = ob
                    nc.vector.tensor_copy(ob[:, ts(0, TC)], st_.pop("ps0")[:])
                    ps = psp.tile([128, TC], F32, tag="b512", bufs=2, name="ops")
                    for p in range(NPAIR):
                        nc.tensor.matmul(
                            ps[:], oT[p][i][:, ts(tt - 4 * i, 128)],
                            wo_sb[:, p * D + 1 * TC:p * D + 2 * TC],
                            start=(p == 0), stop=(p == NPAIR - 1))
                    st_["ps1"] = ps

                def c():
                    ob = st_.pop("ob")
                    nc.vector.tensor_copy(ob[:, ts(1, TC)], st_.pop("ps1")[:])
                    nc.sync.dma_start(out=out_d[ts(tt, 128), :], in_=ob[:])

                return [a, b, c]

            # last chunk: pairs 0-2 accumulated during the chunk, pair 3 +
            # store in the tail (keeps the tail to 8 matmuls + adds)
            last_ob = {}

            def outproj_partial_fills(i, tt):
                def mk(e):
                    def fn():
                        ps = psp.tile([128, TC], F32, tag="b512", bufs=2, name="opp")
                        for p in range(3):
                            nc.tensor.matmul(
                                ps[:], oT[p][i][:, ts(tt - 4 * i, 128)],
                                wo_sb[:, p * D + e * TC:p * D + (e + 1) * TC],
                                start=(p == 0), stop=(p == 2))
                        if tt not in last_ob:
                            last_ob[tt] = mp.tile([128, D], F32, tag="ob",
                                                  bufs=4, name="obL")
                        nc.vector.tensor_copy(last_ob[tt][:, ts(e, TC)], ps[:])
                    return fn
                return [mk(0), mk(1)]

            def outproj_tail(i, tt):
                ob = last_ob[tt]
                for e in range(2):
                    ps = psp.tile([128, TC], F32, tag="b512", bufs=2, name="opt")
                    nc.tensor.matmul(
                        ps[:], oT[3][i][:, ts(tt - 4 * i, 128)],
                        wo_sb[:, 3 * D + e * TC:3 * D + (e + 1) * TC],
                        start=True, stop=True)
                    nc.vector.tensor_add(ob[:, ts(e, TC)], ps[:], ob[:, ts(e, TC)])
                nc.sync.dma_start(out=out_d[ts(tt, 128), :], in_=ob[:])

            # fill queue: (marker, fn); marker <= u fills are forced before
            # unit u's first ST.  99 = pump-only (tail-drained).
            fills = deque()

            def drain_until(u):
                while fills and fills[0][0] <= u:
                    fills.popleft()[1]()

            def pump(n):
                for _ in range(n):
                    if not fills:
                        return
                    fills.popleft()[1]()

            for um in range(1, NTC * NPAIR):
                i, p = um // 4, um % 4
                if p == 0 and i > 0:
                    for sg in range(4 * i, 4 * i + 4):
                        for fn in v_fills(sg):
                            fills.append((um - 1, fn))
                for nm in ("q", "k"):
                    for fn in qk_fills(nm, p, i):
                        fills.append((um - 1, fn))

            # ---- attention unit construction ---------------------------
            scale = 1.0 / math.sqrt(HD)

            def build_unit(i, p):
                if mode == "causal":
                    blocks = []
                    for s_blk in range(4 * i + 4):
                        j = s_blk - 4 * i
                        if j < 0:
                            blocks.append((s_blk, i * TC, TC, False))
                        else:
                            s0 = SB * s_blk
                            toff = s0 if j < 3 else s0 - SB
                            blocks.append((s_blk, toff, TC * (i + 1) - toff, True))
                else:
                    blocks = [(s_blk, i * TC, TC, False)
                              for s_blk in range(NSB) if cls[s_blk, i] != 0]
                state = {"p2": {}, "ot": None}

                def make_st(bi):
                    s_blk, toff, n, diag = blocks[bi]

                    def fn():
                        s0 = SB * s_blk
                        sc, lo = s_blk // 4, SB * (s_blk % 4)
                        tl = toff - i * TC
                        st2 = psp.tile([128, 2 * TC], F32, tag="stAB", bufs=2,
                                       name="st2")
                        nc.tensor.matmul(
                            st2[:, 0:n], kT[p][sc][0:HD, lo:lo + SB],
                            qT[p][i][0:HD, tl:tl + n],
                            start=True, stop=True, tile_position=(0, 0))
                        nc.tensor.matmul(
                            st2[:, TC:TC + n], kT[p][sc][HD:128, lo:lo + SB],
                            qT[p][i][HD:128, tl:tl + n],
                            start=True, stop=True, tile_position=(64, 0))
                        p2 = sp.tile([128, 2 * TC], BF16, tag="pAB", bufs=8,
                                     name="p2")
                        if n == TC:
                            nc.scalar.activation(p2[:], st2[:], AF.Exp, scale=scale)
                        else:
                            st3 = st2[:].rearrange("p (b c) -> p b c", b=2)[:, :, 0:n]
                            p3 = p2[:].rearrange("p (b c) -> p b c", b=2)[:, :, 0:n]
                            nc.scalar.activation(p3, st3, AF.Exp, scale=scale)
                        if mode == "causal" and diag:
                            w_ = s0 + SB - toff
                            for off in (0, TC):
                                nc.gpsimd.affine_select(
                                    out=p2[:, off:off + w_], in_=p2[:, off:off + w_],
                                    compare_op=mybir.AluOpType.is_ge,
                                    fill=0.0, base=toff - s0,
                                    pattern=[[1, w_]], channel_multiplier=-1)
                        elif mode == "general" and cls[s_blk, i] == 2:
                            mmt = sp.tile([SB, TC], BF16, tag="mmask", name="mmt")
                            nc.sync.dma_start(out=mmt[:],
                                              in_=d["mmask"][mixed_idx[(s_blk, i)]])
                            for off in (0, TC):
                                nc.vector.tensor_mul(p2[:, off:off + n],
                                                     p2[:, off:off + n], mmt[:, 0:n])
                        state["p2"][bi] = p2
                    return fn

                def make_pv(bi):
                    s_blk, toff, n, diag = blocks[bi]

                    def fn():
                        if state["ot"] is None:
                            state["ot"] = (
                                psp.tile([HV, TC], F32, tag="ot", bufs=2, name="otA"),
                                psp.tile([HV, TC], F32, tag="ot", bufs=2, name="otB"))
                        otA, otB = state["ot"]
                        p2 = state["p2"].pop(bi)
                        tl = toff - i * TC
                        vv = v_sb[s_blk][:].rearrange("p (h c) -> p h c", c=HV)
                        first, last = bi == 0, bi == len(blocks) - 1
                        nc.tensor.matmul(otA[:, tl:tl + n], vv[:, 2 * p, :],
                                         p2[:, 0:n], start=first, stop=last)
                        nc.tensor.matmul(otB[:, tl:tl + n], vv[:, 2 * p + 1, :],
                                         p2[:, TC:TC + n], start=first, stop=last)
                    return fn

                def epi():
                    otA, otB = state["ot"]
                    # copy PSUM out up-front so the banks free quickly
                    cpA = mp.tile([HV, TC], F32, tag="ocp", bufs=4, name="cpA")
                    cpB = mp.tile([HV, TC], F32, tag="ocp", bufs=4, name="cpB")
                    nc.vector.tensor_copy(cpA[:], otA[:])
                    nc.vector.tensor_copy(cpB[:], otB[:])
                    # both heads' denominators -> partition 0, broadcast, recip
                    dden = mp.tile([1, 2 * TC], F32, tag="dden", name="dden")
                    nc.sync.dma_start(out=dden[:, 0:TC], in_=cpA[HD:HV, :])
                    nc.sync.dma_start(out=dden[:, TC:], in_=cpB[HD:HV, :])
                    rbd = mp.tile([HD, 2 * TC], F32, tag="rbd", name="rbd")
                    nc.gpsimd.partition_broadcast(rbd[:], dden[:])
                    nc.vector.reciprocal_approx_fast(out=rbd[:], in_=rbd[:])
                    nc.vector.tensor_mul(oT[p][i][0:HD, :], cpA[0:HD, :],
                                         rbd[:, 0:TC])
                    stg = mp.tile([HD, TC], BF16, tag="stg", name="stg")
                    nc.vector.tensor_mul(stg[:], cpB[0:HD, :], rbd[:, TC:])
                    nc.gpsimd.dma_start(out=oT[p][i][HD:128, :], in_=stg[:])

                n = len(blocks)
                return ([make_st(b) for b in range(n)],
                        [make_pv(b) for b in range(n)], epi)

            # ---- prologue: pair-0 q/k only (v(0..3) woven into slots) --
            for nm in ("q", "k"):
                for fn in qk_fills(nm, 0, 0):
                    fn()
            if mode != "causal":
                for sg in range(NSB):
                    for fn in v_fills(sg):
                        fn()

            # ---- flat block stream with LAG ----------------------------
            units = [(i, p) for i in range(NTC) for p in range(NPAIR)]
            stream = []
            epis = {}
            for u, (i, p) in enumerate(units):
                st_fns, pv_fns, epi = build_unit(i, p)
                epis[u] = epi
                nb = len(st_fns)
                for b in range(nb):
                    stream.append((st_fns[b], pv_fns[b], u, b == nb - 1))

            def after_chunk(i):
                if i + 2 < NTC:
                    for kind in ("xv", "xq", "xk"):
                        load_x(kind, i + 2)
                um = min(4 * (i + 1) + 2, NTC * NPAIR - 1)
                for tt in range(4 * i, 4 * i + 4):
                    for fn in outproj_fills(i, tt):
                        fills.append((um, fn))

            nblocks = len(stream)
            wo_issued = [False]
            for k in range(nblocks + LAG):
                if k < nblocks:
                    stf, _, u, _ = stream[k]
                    drain_until(u)
                    stf()
                if mode == "causal" and k < 4:
                    for fn in v_fills(k):
                        fn()
                if k == 2 and not wo_issued[0]:
                    wo_issued[0] = True
                    nc.scalar.dma_start(out=wo_sb[:], in_=d["wo"][:])
                    for kind in ("xv", "xq", "xk"):
                        load_x(kind, 1)
                pump(1)
                j = k - LAG
                if j >= 0:
                    _, pvf, u, last = stream[j]
                    pvf()
                    if last:
                        epis[u]()
                        i, p = units[u]
                        if p == NPAIR - 1 and i < NTC - 1:
                            after_chunk(i)
                        if u == NTC * NPAIR - 2:
                            # oT[0..2][last] now all written: stage the last
                            # chunk's pair-0..2 partials (pump-only)
                            for tt in range(4 * (NTC - 1), 4 * NTC):
                                for fn in outproj_partial_fills(NTC - 1, tt):
                                    fills.append((99, fn))
            while fills:
                fills.popleft()[1]()
            for tt in range(4 * (NTC - 1), 4 * NTC):
                outproj_tail(NTC - 1, tt)

    nc.compile()
    return nc


def kernel(**inputs):
    query = np.asarray(inputs["query"], np.float32)
    key = np.asarray(inputs["key"], np.float32)
    value = np.asarray(inputs["value"], np.float32)
    mask = np.asarray(inputs["mask"], bool)
    Wq, bq = np.asarray(inputs["Wq"], np.float32), np.asarray(inputs["bq"], np.float32)
    Wk, bk = np.asarray(inputs["Wk"], np.float32), np.asarray(inputs["bk"], np.float32)
    Wv, bv = np.asarray(inputs["Wv"], np.float32), np.asarray(inputs["bv"], np.float32)
    Wo, bo = np.asarray(inputs["Wo"], np.float32), np.asarray(inputs["bo"], np.float32)

    mode, cls, mixed = _classify_blocks(mask)
    global mixed_idx
    if mode == "general":
        mixed_idx = {blk: n for n, blk in enumerate(mixed)}
        n_mixed = len(mixed)
    else:
        mixed_idx, n_mixed = {}, 0

    key_sig = (mode, tuple(cls.ravel()) if cls is not None else None)
    if key_sig not in _cache:
        _cache[key_sig] = _build(mode, cls, n_mixed)
    nc = _cache[key_sig]

    def xswz(x):
        # [T, D] activation -> [128, NTC, CCH, PC] (chunk-contig per partition)
        xT = np.ascontiguousarray(x.T).astype(ml_dtypes.bfloat16)
        return np.ascontiguousarray(
            xT.reshape(CCH, 128, NTC, PC).transpose(1, 2, 0, 3))

    def wswz_qk(W, sl):
        # [DG, D] shard -> transpose -> [128, NPAIR, CCH*128] pair-contig
        WT = np.ascontiguousarray(W[sl, :].T).astype(ml_dtypes.bfloat16)
        return np.ascontiguousarray(
            WT.reshape(CCH, 128, NPAIR, 128).transpose(1, 2, 0, 3).reshape(
                128, NPAIR, CCH * 128))

    in_maps = []
    xs = {}
    for b in range(B):
        xs[("xq", b)] = xswz(query[b])
        xs[("xk", b)] = xswz(key[b])
        xs[("xv", b)] = xswz(value[b])
    for core in range(NCORE):
        b, g = core // 2, core % 2
        sl = slice(g * DG, (g + 1) * DG)
        WvT = np.ascontiguousarray(Wv[sl, :].T).astype(ml_dtypes.bfloat16)
        WoT = np.ascontiguousarray(Wo[:, sl].T).astype(ml_dtypes.bfloat16)
        im = {
            "xq": xs[("xq", b)], "xk": xs[("xk", b)], "xv": xs[("xv", b)],
            "wq": wswz_qk(Wq, sl),
            "wk": wswz_qk(Wk, sl),
            "wv": np.ascontiguousarray(
                WvT.reshape(CCH, 128, DG).transpose(1, 0, 2).reshape(128, CCH * DG)),
            "wo": np.ascontiguousarray(
                WoT.reshape(NPAIR, 128, D).transpose(1, 0, 2).reshape(128, NPAIR * D)),
            "bq": np.ascontiguousarray(bq[sl].reshape(NPAIR, 128).T),
            "bk": np.ascontiguousarray(bk[sl].reshape(NPAIR, 128).T),
            "bv": np.ascontiguousarray(bv[sl])[None, :].astype(ml_dtypes.bfloat16),
            "ones1": np.ones((1, 128), ml_dtypes.bfloat16),
        }
        if n_mixed:
            mm = np.empty((n_mixed, SB, TC), ml_dtypes.bfloat16)
            for n, (s_blk, i) in enumerate(mixed):
                blk = mask[b, i * TC:(i + 1) * TC, s_blk * SB:(s_blk + 1) * SB]
                mm[n] = (~blk.T).astype(np.float32)
            im["mmask"] = mm
        in_maps.append(im)

    r = run_bass_kernel_spmd(nc, in_maps, core_ids=list(range(NCORE)))
    last_result["exec_time_ns"] = r.exec_time_ns
    last_result["profile_json"] = getattr(r, "profile_json", None)
    last_result["instructions_and_trace"] = getattr(r, "instructions_and_trace", None)
    out = np.empty((B, T, D), np.float32)
    for b in range(B):
        out[b] = r.results[2 * b]["out"] + r.results[2 * b + 1]["out"]
    out += bo[None, None, :]
    return out


# revision 21
# speedup vs baseline: 1.0318x; 1.0003x over previous
"""Cached multi-head attention on 8 TRN2 NeuronCores.

Sharding: core c = 2*b + g handles batch b (of 4) and head-group g (of 2,
8 heads each) -- data parallel on batch x tensor parallel on heads.
Column-parallel Wq/Wk/Wv, row-parallel Wo; the Wo all-reduce (sum of the
two head-group partials per batch) is done on host during the unshard,
along with the bo bias add.

Device schedule (per core): the exp on the Scalar engine (~1.1us per
128x1024 score block) and the PE (~218us of matmul streaming) are kept
continuously busy by emitting attention blocks as a flat stream (ST_k
issued, PV_{k-LAG} trailing) with projection / out-projection matmuls
queued as small fill closures consumed one per block slot.  All weights
and activations are pre-swizzled on the host into SBUF layout so every
DMA is a contiguous 128-descriptor transfer, and startup DMAs are
ordered most-critical-first across the three DGE queues.

Causal masks get a fast path: blocks above the diagonal are skipped,
diagonal blocks use shortened matmuls + gpsimd affine_select zeroing.
Arbitrary masks fall back to per-block skip/plain/mixed classification
with host-shipped multiplicative mask tiles.
"""

import math
from collections import deque

import ml_dtypes
import numpy as np

import concourse.bass as bass
import concourse.mybir as mybir
import concourse.tile as tile
from concourse import bacc
from concourse.bass_utils import run_bass_kernel_spmd

F32 = mybir.dt.float32
BF16 = mybir.dt.bfloat16
AF = mybir.ActivationFunctionType
ts = bass.ts

B, T, D, H = 4, 2048, 1024, 16
HD = D // H          # 64
NCORE = 8
DG = D // 2          # 512 dims per core (8 heads)
NPAIR = 4            # head pairs per core
SB = 128             # s-block size
TC = 512             # attention t-chunk
NTC = T // TC        # 4
NSB = T // SB        # 16
PC = 512             # projection t-chunk (x streaming granularity)
CCH = D // 128       # 8 contraction chunks

LAG = 2              # PV trails ST by this many blocks

_cache = {}
last_result = {}


def _classify_blocks(mask):
    """Per (s_blk, t_chunk) classification, unioned across batches (SPMD)."""
    causal = np.triu(np.ones((T, T), dtype=bool), k=1)
    if all(np.array_equal(mask[b], causal) for b in range(B)):
        return "causal", None, None
    cls = np.zeros((NSB, NTC), dtype=np.int64)
    for s in range(NSB):
        for i in range(NTC):
            per_b_all = [mask[b, i * TC:(i + 1) * TC, s * SB:(s + 1) * SB].all()
                         for b in range(B)]
            per_b_any = [mask[b, i * TC:(i + 1) * TC, s * SB:(s + 1) * SB].any()
                         for b in range(B)]
            if all(per_b_all):
                cls[s, i] = 0
            elif not any(per_b_any):
                cls[s, i] = 1
            else:
                cls[s, i] = 2
    mixed = [(s, i) for s in range(NSB) for i in range(NTC) if cls[s, i] == 2]
    return "general", cls, mixed


def _build(mode, cls, n_mixed):
    nc = bacc.Bacc("TRN2", target_bir_lowering=False, debug=False,
                   num_devices=NCORE)
    d = {}
    # host pre-swizzled layouts: every DMA is contiguous per partition
    for nm in ("xq", "xk", "xv"):
        d[nm] = nc.dram_tensor(nm, [128, NTC, CCH, PC], BF16,
                               kind="ExternalInput").ap()
    for nm in ("wq", "wk"):
        d[nm] = nc.dram_tensor(nm, [128, NPAIR, CCH * 128], BF16,
                               kind="ExternalInput").ap()
    d["wv"] = nc.dram_tensor("wv", [128, CCH * DG], BF16, kind="ExternalInput").ap()
    d["wo"] = nc.dram_tensor("wo", [128, NPAIR * D], BF16, kind="ExternalInput").ap()
    d["bq"] = nc.dram_tensor("bq", [128, NPAIR], F32, kind="ExternalInput").ap()
    d["bk"] = nc.dram_tensor("bk", [128, NPAIR], F32, kind="ExternalInput").ap()
    d["bv"] = nc.dram_tensor("bv", [1, DG], BF16, kind="ExternalInput").ap()
    d["ones1"] = nc.dram_tensor("ones1", [1, 128], BF16, kind="ExternalInput").ap()
    if n_mixed:
        d["mmask"] = nc.dram_tensor("mmask", [n_mixed, SB, TC], BF16,
                                    kind="ExternalInput").ap()
    out_d = nc.dram_tensor("out", [T, D], F32, kind="ExternalOutput").ap()

    with tile.TileContext(nc) as tc:
        with (
            tc.tile_pool(name="persist", bufs=1) as pp,
            tc.tile_pool(name="stream", bufs=2) as sp,
            tc.tile_pool(name="small", bufs=2) as mp,
            tc.tile_pool(name="psum", bufs=2, space="PSUM") as psp,
        ):
            HV = HD + 1  # 65: V columns + ones column per head

            # ---- persistent tiles --------------------------------------
            wv_sb = pp.tile([128, CCH * DG], BF16, tag="wv")
            wq_sb = pp.tile([128, NPAIR * CCH * 128], BF16, tag="wq")
            wk_sb = pp.tile([128, NPAIR * CCH * 128], BF16, tag="wk")
            wo_sb = pp.tile([128, NPAIR * D], BF16, tag="wo")
            bq_sb = pp.tile([128, NPAIR], F32, tag="bq")
            bk_sb = pp.tile([128, NPAIR], F32, tag="bk")
            bv_sb = pp.tile([1, DG], BF16, tag="bv")
            ones1_sb = pp.tile([1, 128], BF16, tag="ones1")
            v_sb = [pp.tile([128, 8 * HV], BF16, tag=f"v{s}", name=f"v{s}")
                    for s in range(NSB)]
            w_sb = {"wq": wq_sb, "wk": wk_sb}

            # ---- startup DMAs, most-critical-first ---------------------
            # scalar (Act) queue is idle until the first exp
            PW = CCH * 128  # columns per pair in wq/wk sbuf layout
            nc.scalar.dma_start(out=wq_sb[:, 0:PW], in_=d["wq"][:, 0])
            nc.scalar.dma_start(out=wk_sb[:, 0:PW], in_=d["wk"][:, 0])
            nc.scalar.dma_start(out=wv_sb[:], in_=d["wv"][:])
            nc.scalar.dma_start(out=bv_sb[:], in_=d["bv"][:])
            nc.scalar.dma_start(out=ones1_sb[:], in_=d["ones1"][:])
            nc.scalar.dma_start(out=wq_sb[:, PW:2 * PW], in_=d["wq"][:, 1])
            nc.scalar.dma_start(out=wk_sb[:, PW:2 * PW], in_=d["wk"][:, 1])
            nc.scalar.dma_start(out=bq_sb[:], in_=d["bq"][:])
            nc.scalar.dma_start(out=bk_sb[:], in_=d["bk"][:])
            for p_ in (2, 3):
                nc.scalar.dma_start(out=wq_sb[:, p_ * PW:(p_ + 1) * PW],
                                    in_=d["wq"][:, p_])
                nc.scalar.dma_start(out=wk_sb[:, p_ * PW:(p_ + 1) * PW],
                                    in_=d["wk"][:, p_])
            # gpsimd queue: warmup tile + the v ones-columns memsets
            warm_sb = pp.tile([128, 512], BF16, tag="warm")
            nc.gpsimd.memset(warm_sb[:], 1.0)
            for s in range(NSB):
                ones_cols = v_sb[s][:].rearrange("p (h c) -> p h c", c=HV)[:, :, HD:HV]
                nc.gpsimd.memset(ones_cols, 1.0)

            # x chunk streaming (sync queue first, then round-robin)
            x_tiles = {}
            _xq_rr = [0]

            def load_x(kind, tau, eng=None, split=False):
                if (kind, tau) in x_tiles:
                    return
                if eng is None:
                    eng = (nc.sync, nc.gpsimd)[_xq_rr[0] % 2]
                    _xq_rr[0] += 1
                xx = sp.tile([128, CCH * PC], BF16, tag=kind, bufs=2,
                             name=f"{kind}{tau}")
                ov = xx[:].rearrange("p (c t) -> p c t", t=PC)
                if split:  # halves so c<4 consumers start sooner
                    eng.dma_start(out=ov[:, 0:4], in_=d[kind][:, tau, 0:4])
                    eng.dma_start(out=ov[:, 4:8], in_=d[kind][:, tau, 4:8])
                else:
                    eng.dma_start(out=ov, in_=d[kind][:, tau])
                x_tiles[(kind, tau)] = xx

            load_x("xq", 0, nc.sync, split=True)
            load_x("xk", 0, nc.gpsimd, split=True)
            load_x("xv", 0, nc.sync, split=True)

            # PE warmup: ramp the p-state while startup DMAs land; results
            # are discarded
            for _ in range(8):
                wps = psp.tile([128, TC], F32, tag="b512", bufs=2, name="wps")
                nc.tensor.matmul(wps[:], warm_sb[:, 0:128], warm_sb[:],
                                 start=True, stop=True)

            qT = [[pp.tile([128, TC], BF16, tag=f"qT{p}_{i}", name=f"qT{p}_{i}")
                   for i in range(NTC)] for p in range(NPAIR)]
            kT = [[pp.tile([128, TC], BF16, tag=f"kT{p}_{i}", name=f"kT{p}_{i}")
                   for i in range(NTC)] for p in range(NPAIR)]
            oT = [[pp.tile([128, TC], BF16, tag=f"oT{p}_{i}", name=f"oT{p}_{i}")
                   for i in range(NTC)] for p in range(NPAIR)]

            # ---- fill closures -----------------------------------------
            def v_fills(sigma):
                tau, u = sigma // 4, sigma % 4
                st_ = {}

                def a():
                    load_x("xv", tau)
                    x = x_tiles[("xv", tau)]
                    ps = psp.tile([128, TC], F32, tag="b512", bufs=2)
                    for c in range(4):
                        nc.tensor.matmul(
                            ps[:], x[:, c * PC + u * SB:c * PC + (u + 1) * SB],
                            wv_sb[:, ts(c, DG)], start=(c == 0), stop=False)
                    st_["ps"] = ps

                def b():
                    x = x_tiles[("xv", tau)]
                    ps = st_.pop("ps")
                    for c in range(4, CCH):
                        nc.tensor.matmul(
                            ps[:], x[:, c * PC + u * SB:c * PC + (u + 1) * SB],
                            wv_sb[:, ts(c, DG)], start=False, stop=False)
                    nc.tensor.matmul(ps[:], ones1_sb[:], bv_sb[:],
                                     start=False, stop=True)
                    vdst = v_sb[sigma][:].rearrange("p (h c) -> p h c", c=HV)[:, :, 0:HD]
                    nc.vector.tensor_copy(vdst, ps[:].rearrange("p (h c) -> p h c", c=HD))

                return [a, b]

            def qk_fills(nm, p, i):
                st_ = {}
                dst = qT if nm == "q" else kT
                bias = bq_sb if nm == "q" else bk_sb
                xkind = "xq" if nm == "q" else "xk"
                w = w_sb["w" + nm]

                def a():
                    load_x(xkind, i)
                    xx = x_tiles[(xkind, i)]
                    ps = psp.tile([128, TC], F32, tag="b512", bufs=2)
                    for c in range(4):
                        nc.tensor.matmul(
                            ps[:], w[:, (p * CCH + c) * 128:(p * CCH + c + 1) * 128],
                            xx[:, ts(c, PC)], start=(c == 0), stop=False)
                    st_["ps"] = ps

                def b():
                    xx = x_tiles[(xkind, i)]
                    ps = st_.pop("ps")
                    for c in range(4, CCH):
                        nc.tensor.matmul(
                            ps[:], w[:, (p * CCH + c) * 128:(p * CCH + c + 1) * 128],
                            xx[:, ts(c, PC)], start=False, stop=(c == CCH - 1))
                    nc.vector.tensor_scalar(
                        out=dst[p][i][:], in0=ps[:],
                        scalar1=bias[:, p:p + 1], scalar2=None,
                        op0=mybir.AluOpType.add)

                return [a, b]

            def outproj_fills(i, tt):
                # chunks 0..2: full 4-pair accumulation per (tt, e)
                st_ = {}

                def a():
                    ps = psp.tile([128, TC], F32, tag="b512", bufs=2, name="ops")
                    for p in range(NPAIR):
                        nc.tensor.matmul(
                            ps[:], oT[p][i][:, ts(tt - 4 * i, 128)],
                            wo_sb[:, p * D + 0 * TC:p * D + 1 * TC],
                            start=(p == 0), stop=(p == NPAIR - 1))
                    st_["ps0"] = ps

                def b():
                    ob = mp.tile([128, D], F32, tag="ob", bufs=4, name="ob")
                    st_["ob"] = ob
                    nc.vector.tensor_copy(ob[:, ts(0, TC)], st_.pop("ps0")[:])
                    ps = psp.tile([128, TC], F32, tag="b512", bufs=2, name="ops")
                    for p in range(NPAIR):
                        nc.tensor.matmul(
                            ps[:], oT[p][i][:, ts(tt - 4 * i, 128)],
                            wo_sb[:, p * D + 1 * TC:p * D + 2 * TC],
                            start=(p == 0), stop=(p == NPAIR - 1))
                    st_["ps1"] = ps

                def c():
                    ob = st_.pop("ob")
                    nc.vector.tensor_copy(ob[:, ts(1, TC)], st_.pop("ps1")[:])
                    nc.sync.dma_start(out=out_d[ts(tt, 128), :], in_=ob[:])

                return [a, b, c]

            # last chunk: pairs 0-2 accumulated during the chunk, pair 3 +
            # store in the tail (keeps the tail to 8 matmuls + adds)
            last_ob = {}

            def outproj_partial_fills(i, tt):
                def mk(e):
                    def fn():
                        ps = psp.tile([128, TC], F32, tag="b512", bufs=2, name="opp")
                        for p in range(3):
                            nc.tensor.matmul(
                                ps[:], oT[p][i][:, ts(tt - 4 * i, 128)],
                                wo_sb[:, p * D + e * TC:p * D + (e + 1) * TC],
                                start=(p == 0), stop=(p == 2))
                        if tt not in last_ob:
                            last_ob[tt] = mp.tile([128, D], F32, tag="ob",
                                                  bufs=4, name="obL")
                        nc.vector.tensor_copy(last_ob[tt][:, ts(e, TC)], ps[:])
                    return fn
                return [mk(0), mk(1)]

            def outproj_tail(i, tt):
                ob = last_ob[tt]
                for e in range(2):
                    ps = psp.tile([128, TC], F32, tag="b512", bufs=2, name="opt")
                    nc.tensor.matmul(
                        ps[:], oT[3][i][:, ts(tt - 4 * i, 128)],
                        wo_sb[:, 3 * D + e * TC:3 * D + (e + 1) * TC],
                        start=True, stop=True)
                    nc.vector.tensor_add(ob[:, ts(e, TC)], ps[:], ob[:, ts(e, TC)])
                nc.sync.dma_start(out=out_d[ts(tt, 128), :], in_=ob[:])

            # fill queue: (marker, fn); marker <= u fills are forced before
            # unit u's first ST.  99 = pump-only (tail-drained).
            fills = deque()

            def drain_until(u):
                while fills and fills[0][0] <= u:
                    fills.popleft()[1]()

            def pump(n):
                for _ in range(n):
                    if not fills:
                        return
                    fills.popleft()[1]()

            for um in range(1, NTC * NPAIR):
                i, p = um // 4, um % 4
                if p == 0 and i > 0:
                    for sg in range(4 * i, 4 * i + 4):
                        for fn in v_fills(sg):
                            fills.append((um, fn))
                for nm in ("q", "k"):
                    for fn in qk_fills(nm, p, i):
                        fills.append((um, fn))

            # ---- attention unit construction ---------------------------
            scale = 1.0 / math.sqrt(HD)

            def build_unit(i, p):
                if mode == "causal":
                    blocks = []
                    for s_blk in range(4 * i + 4):
                        j = s_blk - 4 * i
                        if j < 0:
                            blocks.append((s_blk, i * TC, TC, False))
                        else:
                            s0 = SB * s_blk
                            toff = s0 if j < 3 else s0 - SB
                            blocks.append((s_blk, toff, TC * (i + 1) - toff, True))
                else:
                    blocks = [(s_blk, i * TC, TC, False)
                              for s_blk in range(NSB) if cls[s_blk, i] != 0]
                state = {"p2": {}, "ot": None}

                def make_st(bi):
                    s_blk, toff, n, diag = blocks[bi]

                    def fn():
                        s0 = SB * s_blk
                        sc, lo = s_blk // 4, SB * (s_blk % 4)
                        tl = toff - i * TC
                        st2 = psp.tile([128, 2 * TC], F32, tag="stAB", bufs=2,
                                       name="st2")
                        nc.tensor.matmul(
                            st2[:, 0:n], kT[p][sc][0:HD, lo:lo + SB],
                            qT[p][i][0:HD, tl:tl + n],
                            start=True, stop=True, tile_position=(0, 0))
                        nc.tensor.matmul(
                            st2[:, TC:TC + n], kT[p][sc][HD:128, lo:lo + SB],
                            qT[p][i][HD:128, tl:tl + n],
                            start=True, stop=True, tile_position=(64, 0))
                        p2 = sp.tile([128, 2 * TC], BF16, tag="pAB", bufs=8,
                                     name="p2")
                        if n == TC:
                            nc.scalar.activation(p2[:], st2[:], AF.Exp, scale=scale)
                        else:
                            st3 = st2[:].rearrange("p (b c) -> p b c", b=2)[:, :, 0:n]
                            p3 = p2[:].rearrange("p (b c) -> p b c", b=2)[:, :, 0:n]
                            nc.scalar.activation(p3, st3, AF.Exp, scale=scale)
                        if mode == "causal" and diag:
                            w_ = s0 + SB - toff
                            for off in (0, TC):
                                nc.gpsimd.affine_select(
                                    out=p2[:, off:off + w_], in_=p2[:, off:off + w_],
                                    compare_op=mybir.AluOpType.is_ge,
                                    fill=0.0, base=toff - s0,
                                    pattern=[[1, w_]], channel_multiplier=-1)
                        elif mode == "general" and cls[s_blk, i] == 2:
                            mmt = sp.tile([SB, TC], BF16, tag="mmask", name="mmt")
                            nc.sync.dma_start(out=mmt[:],
                                              in_=d["mmask"][mixed_idx[(s_blk, i)]])
                            for off in (0, TC):
                                nc.vector.tensor_mul(p2[:, off:off + n],
                                                     p2[:, off:off + n], mmt[:, 0:n])
                        state["p2"][bi] = p2
                    return fn

                def make_pv(bi):
                    s_blk, toff, n, diag = blocks[bi]

                    def fn():
                        if state["ot"] is None:
                            state["ot"] = (
                                psp.tile([HV, TC], F32, tag="ot", bufs=2, name="otA"),
                                psp.tile([HV, TC], F32, tag="ot", bufs=2, name="otB"))
                        otA, otB = state["ot"]
                        p2 = state["p2"].pop(bi)
                        tl = toff - i * TC
                        vv = v_sb[s_blk][:].rearrange("p (h c) -> p h c", c=HV)
                        first, last = bi == 0, bi == len(blocks) - 1
                        nc.tensor.matmul(otA[:, tl:tl + n], vv[:, 2 * p, :],
                                         p2[:, 0:n], start=first, stop=last)
                        nc.tensor.matmul(otB[:, tl:tl + n], vv[:, 2 * p + 1, :],
                                         p2[:, TC:TC + n], start=first, stop=last)
                    return fn

                def epi():
                    otA, otB = state["ot"]
                    fast = (i == NTC - 1 and p == NPAIR - 1)
                    dq = nc.scalar if fast else nc.sync
                    # copy PSUM out up-front so the banks free quickly
                    cpA = mp.tile([HV, TC], F32, tag="ocp", bufs=4, name="cpA")
                    cpB = mp.tile([HV, TC], F32, tag="ocp", bufs=4, name="cpB")
                    nc.vector.tensor_copy(cpA[:], otA[:])
                    nc.vector.tensor_copy(cpB[:], otB[:])
                    # both heads' denominators -> partition 0, broadcast, recip
                    dden = mp.tile([1, 2 * TC], F32, tag="dden", name="dden")
                    dq.dma_start(out=dden[:, 0:TC], in_=cpA[HD:HV, :])
                    dq.dma_start(out=dden[:, TC:], in_=cpB[HD:HV, :])
                    rbd = mp.tile([HD, 2 * TC], F32, tag="rbd", name="rbd")
                    if fast:
                        # PE-based broadcast: gpsimd is congested at the tail
                        ones_f = mp.tile([1, HD], F32, tag="onesf", name="onesf")
                        nc.vector.tensor_copy(ones_f[:], ones1_sb[0:1, 0:HD])
                        for hh in range(2):
                            bps = psp.tile([128, TC], F32, tag="b512", bufs=2,
                                           name="bps")
                            nc.tensor.matmul(bps[0:HD, :], ones_f[:],
                                             dden[:, hh * TC:(hh + 1) * TC],
                                             start=True, stop=True)
                            nc.vector.tensor_copy(rbd[:, hh * TC:(hh + 1) * TC],
                                                  bps[0:HD, :])
                    else:
                        nc.gpsimd.partition_broadcast(rbd[:], dden[:])
                    nc.vector.reciprocal_approx_fast(out=rbd[:], in_=rbd[:])
                    nc.vector.tensor_mul(oT[p][i][0:HD, :], cpA[0:HD, :],
                                         rbd[:, 0:TC])
                    stg = mp.tile([HD, TC], BF16, tag="stg", name="stg")
                    nc.vector.tensor_mul(stg[:], cpB[0:HD, :], rbd[:, TC:])
                    dq.dma_start(out=oT[p][i][HD:128, :], in_=stg[:])

                n = len(blocks)
                return ([make_st(b) for b in range(n)],
                        [make_pv(b) for b in range(n)], epi)

            # ---- prologue: pair-0 q/k only (v(0..3) woven into slots) --
            for nm in ("q", "k"):
                for fn in qk_fills(nm, 0, 0):
                    fn()
            if mode != "causal":
                for sg in range(NSB):
                    for fn in v_fills(sg):
                        fn()

            # ---- flat block stream with LAG ----------------------------
            units = [(i, p) for i in range(NTC) for p in range(NPAIR)]
            stream = []
            epis = {}
            for u, (i, p) in enumerate(units):
                st_fns, pv_fns, epi = build_unit(i, p)
                epis[u] = epi
                nb = len(st_fns)
                for b in range(nb):
                    stream.append((st_fns[b], pv_fns[b], u, b == nb - 1))

            def after_chunk(i):
                if i + 2 < NTC:
                    for kind in ("xv", "xq", "xk"):
                        load_x(kind, i + 2)
                um = min(4 * (i + 1) + 2, NTC * NPAIR - 1)
                for tt in range(4 * i, 4 * i + 4):
                    for fn in outproj_fills(i, tt):
                        fills.append((um, fn))

            nblocks = len(stream)
            wo_issued = [False]
            for k in range(nblocks + LAG):
                if k < nblocks:
                    stf, _, u, _ = stream[k]
                    drain_until(u)
                    stf()
                if mode == "causal" and k < 4:
                    for fn in v_fills(k):
                        fn()
                if k == 2 and not wo_issued[0]:
                    wo_issued[0] = True
                    nc.scalar.dma_start(out=wo_sb[:], in_=d["wo"][:])
                    for kind in ("xv", "xq", "xk"):
                        load_x(kind, 1)
                pump(2 if len(fills) > 20 else 1)
                j = k - LAG
                if j >= 0:
                    _, pvf, u, last = stream[j]
                    pvf()
                    if last:
                        epis[u]()
                        i, p = units[u]
                        if p == NPAIR - 1 and i < NTC - 1:
                            after_chunk(i)
                        if u == NTC * NPAIR - 2:
                            # oT[0..2][last] now all written: stage the last
                            # chunk's pair-0..2 partials (pump-only)
                            for tt in range(4 * (NTC - 1), 4 * NTC):
                                for fn in outproj_partial_fills(NTC - 1, tt):
                                    fills.append((99, fn))
            while fills:
                fills.popleft()[1]()
            for tt in range(4 * (NTC - 1), 4 * NTC):
                outproj_tail(NTC - 1, tt)

    nc.compile()
    return nc


def kernel(**inputs):
    query = np.asarray(inputs["query"], np.float32)
    key = np.asarray(inputs["key"], np.float32)
    value = np.asarray(inputs["value"], np.float32)
    mask = np.asarray(inputs["mask"], bool)
    Wq, bq = np.asarray(inputs["Wq"], np.float32), np.asarray(inputs["bq"], np.float32)
    Wk, bk = np.asarray(inputs["Wk"], np.float32), np.asarray(inputs["bk"], np.float32)
    Wv, bv = np.asarray(inputs["Wv"], np.float32), np.asarray(inputs["bv"], np.float32)
    Wo, bo = np.asarray(inputs["Wo"], np.float32), np.asarray(inputs["bo"], np.float32)

    mode, cls, mixed = _classify_blocks(mask)
    global mixed_idx
    if mode == "general":
        mixed_idx = {blk: n for n, blk in enumerate(mixed)}
        n_mixed = len(mixed)
    else:
        mixed_idx, n_mixed = {}, 0

    key_sig = (mode, tuple(cls.ravel()) if cls is not None else None)
    if key_sig not in _cache:
        _cache[key_sig] = _build(mode, cls, n_mixed)
    nc = _cache[key_sig]

    def xswz(x):
        # [T, D] activation -> [128, NTC, CCH, PC] (chunk-contig per partition)
        xT = np.ascontiguousarray(x.T).astype(ml_dtypes.bfloat16)
        return np.ascontiguousarray(
            xT.reshape(CCH, 128, NTC, PC).transpose(1, 2, 0, 3))

    def wswz_qk(W, sl):
        # [DG, D] shard -> transpose -> [128, NPAIR, CCH*128] pair-contig
        WT = np.ascontiguousarray(W[sl, :].T).astype(ml_dtypes.bfloat16)
        return np.ascontiguousarray(
            WT.reshape(CCH, 128, NPAIR, 128).transpose(1, 2, 0, 3).reshape(
                128, NPAIR, CCH * 128))

    in_maps = []
    xs = {}
    for b in range(B):
        xs[("xq", b)] = xswz(query[b])
        xs[("xk", b)] = xswz(key[b])
        xs[("xv", b)] = xswz(value[b])
    for core in range(NCORE):
        b, g = core // 2, core % 2
        sl = slice(g * DG, (g + 1) * DG)
        WvT = np.ascontiguousarray(Wv[sl, :].T).astype(ml_dtypes.bfloat16)
        WoT = np.ascontiguousarray(Wo[:, sl].T).astype(ml_dtypes.bfloat16)
        im = {
            "xq": xs[("xq", b)], "xk": xs[("xk", b)], "xv": xs[("xv", b)],
            "wq": wswz_qk(Wq, sl),
            "wk": wswz_qk(Wk, sl),
            "wv": np.ascontiguousarray(
                WvT.reshape(CCH, 128, DG).transpose(1, 0, 2).reshape(128, CCH * DG)),
            "wo": np.ascontiguousarray(
                WoT.reshape(NPAIR, 128, D).transpose(1, 0, 2).reshape(128, NPAIR * D)),
            "bq": np.ascontiguousarray(bq[sl].reshape(NPAIR, 128).T),
            "bk": np.ascontiguousarray(bk[sl].reshape(NPAIR, 128).T),
            "bv": np.ascontiguousarray(bv[sl])[None, :].astype(ml_dtypes.bfloat16),
            "ones1": np.ones((1, 128), ml_dtypes.bfloat16),
        }
        if n_mixed:
            mm = np.empty((n_mixed, SB, TC), ml_dtypes.bfloat16)
            for n, (s_blk, i) in enumerate(mixed):
                blk = mask[b, i * TC:(i + 1) * TC, s_blk * SB:(s_blk + 1) * SB]
                mm[n] = (~blk.T).astype(np.float32)
            im["mmask"] = mm
        in_maps.append(im)

    r = run_bass_kernel_spmd(nc, in_maps, core_ids=list(range(NCORE)))
    last_result["exec_time_ns"] = r.exec_time_ns
    last_result["profile_json"] = getattr(r, "profile_json", None)
    last_result["instructions_and_trace"] = getattr(r, "instructions_and_trace", None)
    out = np.empty((B, T, D), np.float32)
    for b in range(B):
        out[b] = r.results[2 * b]["out"] + r.results[2 * b + 1]["out"]
    out += bo[None, None, :]
    return out


# revision 33
# speedup vs baseline: 1.0600x; 1.0274x over previous
"""Cached multi-head attention on 8 TRN2 NeuronCores.

Sharding: core c = 2*b + g handles batch b (of 4) and head-group g (of 2,
8 heads each) -- data parallel on batch x tensor parallel on heads.
Column-parallel Wq/Wk/Wv, row-parallel Wo; the Wo all-reduce (sum of the
two head-group partials per batch) is done on host during the unshard,
along with the bo bias add.

Device schedule (per core): the exp on the Scalar engine (~1.1us per
128x1024 score block) and the PE (~218us of matmul streaming) are kept
continuously busy by emitting attention blocks as a flat stream (ST_k
issued, PV_{k-LAG} trailing) with projection / out-projection matmuls
queued as small fill closures consumed one per block slot.  All weights
and activations are pre-swizzled on the host into SBUF layout so every
DMA is a contiguous 128-descriptor transfer, and startup DMAs are
ordered most-critical-first across the three DGE queues.

Causal masks get a fast path: blocks above the diagonal are skipped,
diagonal blocks use shortened matmuls + gpsimd affine_select zeroing.
Arbitrary masks fall back to per-block skip/plain/mixed classification
with host-shipped multiplicative mask tiles.
"""

import bisect
import math

import ml_dtypes
import numpy as np

import concourse.bass as bass
import concourse.mybir as mybir
import concourse.tile as tile
from concourse import bacc
from concourse.bass_utils import run_bass_kernel_spmd

F32 = mybir.dt.float32
BF16 = mybir.dt.bfloat16
AF = mybir.ActivationFunctionType
ts = bass.ts

B, T, D, H = 4, 2048, 1024, 16
HD = D // H          # 64
NCORE = 8
DG = D // 2          # 512 dims per core (8 heads)
NPAIR = 4            # head pairs per core
SB = 128             # s-block size
TC = 512             # attention t-chunk
NTC = T // TC        # 4
NSB = T // SB        # 16
PC = 512             # projection t-chunk (x streaming granularity)
CCH = D // 128       # 8 contraction chunks

LAG = 4              # PV trails ST by this many blocks

_cache = {}
last_result = {}


def _classify_blocks(mask):
    """Per (s_blk, t_chunk) classification, unioned across batches (SPMD)."""
    causal = np.triu(np.ones((T, T), dtype=bool), k=1)
    if all(np.array_equal(mask[b], causal) for b in range(B)):
        return "causal", None, None
    cls = np.zeros((NSB, NTC), dtype=np.int64)
    for s in range(NSB):
        for i in range(NTC):
            per_b_all = [mask[b, i * TC:(i + 1) * TC, s * SB:(s + 1) * SB].all()
                         for b in range(B)]
            per_b_any = [mask[b, i * TC:(i + 1) * TC, s * SB:(s + 1) * SB].any()
                         for b in range(B)]
            if all(per_b_all):
                cls[s, i] = 0
            elif not any(per_b_any):
                cls[s, i] = 1
            else:
                cls[s, i] = 2
    mixed = [(s, i) for s in range(NSB) for i in range(NTC) if cls[s, i] == 2]
    return "general", cls, mixed


def _build(mode, cls, n_mixed):
    nc = bacc.Bacc("TRN2", target_bir_lowering=False, debug=False,
                   num_devices=NCORE)
    d = {}
    # host pre-swizzled layouts: every DMA is contiguous per partition
    for nm in ("xq", "xk", "xv"):
        d[nm] = nc.dram_tensor(nm, [128, NTC, CCH, PC], BF16,
                               kind="ExternalInput").ap()
    for nm in ("wq", "wk"):
        d[nm] = nc.dram_tensor(nm, [128, NPAIR, CCH * 128], BF16,
                               kind="ExternalInput").ap()
    d["wv"] = nc.dram_tensor("wv", [128, CCH * DG], BF16, kind="ExternalInput").ap()
    d["wo"] = nc.dram_tensor("wo", [128, NPAIR * D], BF16, kind="ExternalInput").ap()
    d["bq"] = nc.dram_tensor("bq", [128, NPAIR], F32, kind="ExternalInput").ap()
    d["bk"] = nc.dram_tensor("bk", [128, NPAIR], F32, kind="ExternalInput").ap()
    d["bv"] = nc.dram_tensor("bv", [1, DG], BF16, kind="ExternalInput").ap()
    d["ones1"] = nc.dram_tensor("ones1", [1, 128], BF16, kind="ExternalInput").ap()
    if n_mixed:
        d["mmask"] = nc.dram_tensor("mmask", [n_mixed, SB, TC], BF16,
                                    kind="ExternalInput").ap()
    out_d = nc.dram_tensor("out", [T, D], F32, kind="ExternalOutput").ap()

    with tile.TileContext(nc) as tc:
        with (
            tc.tile_pool(name="persist", bufs=1) as pp,
            tc.tile_pool(name="stream", bufs=2) as sp,
            tc.tile_pool(name="small", bufs=2) as mp,
            tc.tile_pool(name="psum", bufs=2, space="PSUM") as psp,
        ):
            HV = HD + 1  # 65: V columns + ones column per head

            # ---- persistent tiles --------------------------------------
            wv_sb = pp.tile([128, CCH * DG], BF16, tag="wv")
            wq_sb = pp.tile([128, NPAIR * CCH * 128], BF16, tag="wq")
            wk_sb = pp.tile([128, NPAIR * CCH * 128], BF16, tag="wk")
            wo_sb = pp.tile([128, NPAIR * D], BF16, tag="wo")
            bq_sb = pp.tile([128, NPAIR], F32, tag="bq")
            bk_sb = pp.tile([128, NPAIR], F32, tag="bk")
            bv_sb = pp.tile([1, DG], BF16, tag="bv")
            ones1_sb = pp.tile([1, 128], BF16, tag="ones1")
            v_sb = [pp.tile([128, 8 * HV], BF16, tag=f"v{s}", name=f"v{s}")
                    for s in range(NSB)]
            w_sb = {"wq": wq_sb, "wk": wk_sb}

            # ---- startup DMAs: balance the three ~100GB/s DGE queues ---
            # scalar (Act): all wq/wk pairs + small tensors (~2MB)
            PW = CCH * 128  # columns per pair in wq/wk sbuf layout
            nc.scalar.dma_start(out=wq_sb[:, 0:PW], in_=d["wq"][:, 0])
            nc.scalar.dma_start(out=wk_sb[:, 0:PW], in_=d["wk"][:, 0])
            nc.scalar.dma_start(out=bv_sb[:], in_=d["bv"][:])
            nc.scalar.dma_start(out=ones1_sb[:], in_=d["ones1"][:])
            nc.scalar.dma_start(out=bq_sb[:], in_=d["bq"][:])
            nc.scalar.dma_start(out=bk_sb[:], in_=d["bk"][:])
            for p_ in (1, 2, 3):
                nc.scalar.dma_start(out=wq_sb[:, p_ * PW:(p_ + 1) * PW],
                                    in_=d["wq"][:, p_])
                nc.scalar.dma_start(out=wk_sb[:, p_ * PW:(p_ + 1) * PW],
                                    in_=d["wk"][:, p_])
            # gpsimd queue: warmup tile memset first (gates PE warmup)
            warm_sb = pp.tile([128, 512], BF16, tag="warm")
            nc.gpsimd.memset(warm_sb[:], 1.0)

            # x chunk streaming
            x_tiles = {}
            _xq_rr = [0]

            def load_x(kind, tau, eng=None):
                if (kind, tau) in x_tiles:
                    return
                if eng is None:
                    eng = (nc.sync, nc.gpsimd)[_xq_rr[0] % 2]
                    _xq_rr[0] += 1
                xx = sp.tile([128, CCH * PC], BF16, tag=kind, bufs=2,
                             name=f"{kind}{tau}")
                eng.dma_start(
                    out=xx[:].rearrange("p (c t) -> p c t", t=PC),
                    in_=d[kind][:, tau])
                x_tiles[(kind, tau)] = xx

            # sync: xq0 then xv0; gpsimd: xk0 then wv (~2MB each queue)
            load_x("xq", 0, nc.sync)
            load_x("xk", 0, nc.gpsimd)
            load_x("xv", 0, nc.sync)
            nc.gpsimd.dma_start(out=wv_sb[:], in_=d["wv"][:])
            for s in range(NSB):
                ones_cols = v_sb[s][:].rearrange("p (h c) -> p h c", c=HV)[:, :, HD:HV]
                nc.gpsimd.memset(ones_cols, 1.0)

            # PE warmup: ramp the p-state while startup DMAs land; results
            # are discarded
            for _ in range(8):
                wps = psp.tile([128, TC], F32, tag="b512", bufs=2, name="wps")
                nc.tensor.matmul(wps[:], warm_sb[:, 0:128], warm_sb[:],
                                 start=True, stop=True)

            qT = [[pp.tile([128, TC], BF16, tag=f"qT{p}_{i}", name=f"qT{p}_{i}")
                   for i in range(NTC)] for p in range(NPAIR)]
            kT = [[pp.tile([128, TC], BF16, tag=f"kT{p}_{i}", name=f"kT{p}_{i}")
                   for i in range(NTC)] for p in range(NPAIR)]
            oT = [[pp.tile([128, TC], BF16, tag=f"oT{p}_{i}", name=f"oT{p}_{i}")
                   for i in range(NTC)] for p in range(NPAIR)]

            # ---- fill closures -----------------------------------------
            def v_fills(sigma):
                tau, u = sigma // 4, sigma % 4
                st_ = {}

                def a():
                    load_x("xv", tau)
                    x = x_tiles[("xv", tau)]
                    ps = psp.tile([128, TC], F32, tag="b512", bufs=2)
                    for c in range(4):
                        nc.tensor.matmul(
                            ps[:], x[:, c * PC + u * SB:c * PC + (u + 1) * SB],
                            wv_sb[:, ts(c, DG)], start=(c == 0), stop=False)
                    st_["ps"] = ps

                def b():
                    x = x_tiles[("xv", tau)]
                    ps = st_.pop("ps")
                    for c in range(4, CCH):
                        nc.tensor.matmul(
                            ps[:], x[:, c * PC + u * SB:c * PC + (u + 1) * SB],
                            wv_sb[:, ts(c, DG)], start=False, stop=False)
                    nc.tensor.matmul(ps[:], ones1_sb[:], bv_sb[:],
                                     start=False, stop=True)
                    vdst = v_sb[sigma][:].rearrange("p (h c) -> p h c", c=HV)[:, :, 0:HD]
                    nc.vector.tensor_copy(vdst, ps[:].rearrange("p (h c) -> p h c", c=HD))

                return [a, b]

            def qk_fills(nm, p, i):
                st_ = {}
                dst = qT if nm == "q" else kT
                bias = bq_sb if nm == "q" else bk_sb
                xkind = "xq" if nm == "q" else "xk"
                w = w_sb["w" + nm]

                def a():
                    load_x(xkind, i)
                    xx = x_tiles[(xkind, i)]
                    ps = psp.tile([128, TC], F32, tag="b512", bufs=2)
                    for c in range(4):
                        nc.tensor.matmul(
                            ps[:], w[:, (p * CCH + c) * 128:(p * CCH + c + 1) * 128],
                            xx[:, ts(c, PC)], start=(c == 0), stop=False)
                    st_["ps"] = ps

                def b():
                    xx = x_tiles[(xkind, i)]
                    ps = st_.pop("ps")
                    for c in range(4, CCH):
                        nc.tensor.matmul(
                            ps[:], w[:, (p * CCH + c) * 128:(p * CCH + c + 1) * 128],
                            xx[:, ts(c, PC)], start=False, stop=(c == CCH - 1))
                    nc.vector.tensor_scalar(
                        out=dst[p][i][:], in0=ps[:],
                        scalar1=bias[:, p:p + 1], scalar2=None,
                        op0=mybir.AluOpType.add)

                return [a, b]

            def outproj_fills(i, tt):
                # chunks 0..2: full 4-pair accumulation per (tt, e)
                st_ = {}

                def a():
                    ps = psp.tile([128, TC], F32, tag="b512", bufs=2, name="ops")
                    for p in range(NPAIR):
                        nc.tensor.matmul(
                            ps[:], oT[p][i][:, ts(tt - 4 * i, 128)],
                            wo_sb[:, p * D + 0 * TC:p * D + 1 * TC],
                            start=(p == 0), stop=(p == NPAIR - 1))
                    st_["ps0"] = ps

                def b():
                    ob = mp.tile([128, D], F32, tag="ob", bufs=4, name="ob")
                    st_["ob"] = ob
                    nc.vector.tensor_copy(ob[:, ts(0, TC)], st_.pop("ps0")[:])
                    ps = psp.tile([128, TC], F32, tag="b512", bufs=2, name="ops")
                    for p in range(NPAIR):
                        nc.tensor.matmul(
                            ps[:], oT[p][i][:, ts(tt - 4 * i, 128)],
                            wo_sb[:, p * D + 1 * TC:p * D + 2 * TC],
                            start=(p == 0), stop=(p == NPAIR - 1))
                    st_["ps1"] = ps

                def c():
                    ob = st_.pop("ob")
                    nc.vector.tensor_copy(ob[:, ts(1, TC)], st_.pop("ps1")[:])
                    nc.sync.dma_start(out=out_d[ts(tt, 128), :], in_=ob[:])

                return [a, b, c]

            # last chunk: pairs 0-2 accumulated during the chunk, pair 3 +
            # store in the tail (keeps the tail to 8 matmuls + adds)
            last_ob = {}

            def outproj_partial_fills(i, tt):
                def mk(e):
                    def fn():
                        ps = psp.tile([128, TC], F32, tag="b512", bufs=2, name="opp")
                        for p in range(3):
                            nc.tensor.matmul(
                                ps[:], oT[p][i][:, ts(tt - 4 * i, 128)],
                                wo_sb[:, p * D + e * TC:p * D + (e + 1) * TC],
                                start=(p == 0), stop=(p == 2))
                        if tt not in last_ob:
                            last_ob[tt] = mp.tile([128, D], F32, tag="ob",
                                                  bufs=4, name="obL")
                        nc.vector.tensor_copy(last_ob[tt][:, ts(e, TC)], ps[:])
                    return fn
                return [mk(0), mk(1)]

            def outproj_tail(i, tt):
                ob = last_ob[tt]
                for e in range(2):
                    ps = psp.tile([128, TC], F32, tag="b512", bufs=2, name="opt")
                    nc.tensor.matmul(
                        ps[:], oT[3][i][:, ts(tt - 4 * i, 128)],
                        wo_sb[:, 3 * D + e * TC:3 * D + (e + 1) * TC],
                        start=True, stop=True)
                    nc.vector.tensor_add(ob[:, ts(e, TC)], ps[:], ob[:, ts(e, TC)])
                nc.sync.dma_start(out=out_d[ts(tt, 128), :], in_=ob[:])

            # fill queue: (marker, seq, fn) kept sorted by marker; fills with
            # marker <= u are forced before unit u's first ST.  99 =
            # pump-only (tail-drained).
            fills = []
            _fseq = [0]

            def fpush(marker, fn):
                bisect.insort(fills, (marker, _fseq[0], fn))
                _fseq[0] += 1

            def drain_until(u):
                while fills and fills[0][0] <= u:
                    fills.pop(0)[2]()

            def pump(n):
                for _ in range(n):
                    if not fills:
                        return
                    fills.pop(0)[2]()

            for um in range(1, NTC * NPAIR):
                i, p = um // 4, um % 4
                if p == 0 and i > 0:
                    for sg in range(4 * i, 4 * i + 4):
                        for fn in v_fills(sg):
                            fpush(um, fn)
                for nm in ("q", "k"):
                    for fn in qk_fills(nm, p, i):
                        fpush(um, fn)

            # ---- attention unit construction ---------------------------
            scale = 1.0 / math.sqrt(HD)

            def build_unit(i, p):
                if mode == "causal":
                    blocks = []
                    for s_blk in range(4 * i + 4):
                        j = s_blk - 4 * i
                        if j < 0:
                            blocks.append((s_blk, i * TC, TC, False))
                        else:
                            s0 = SB * s_blk
                            toff = s0 if j < 3 else s0 - SB
                            blocks.append((s_blk, toff, TC * (i + 1) - toff, True))
                else:
                    blocks = [(s_blk, i * TC, TC, False)
                              for s_blk in range(NSB) if cls[s_blk, i] != 0]
                state = {"p2": {}, "ot": None}

                def make_st(bi):
                    s_blk, toff, n, diag = blocks[bi]

                    def fn():
                        s0 = SB * s_blk
                        sc, lo = s_blk // 4, SB * (s_blk % 4)
                        tl = toff - i * TC
                        st2 = psp.tile([128, 2 * TC], F32, tag="stAB", bufs=2,
                                       name="st2")
                        nc.tensor.matmul(
                            st2[:, 0:n], kT[p][sc][0:HD, lo:lo + SB],
                            qT[p][i][0:HD, tl:tl + n],
                            start=True, stop=True, tile_position=(0, 0))
                        nc.tensor.matmul(
                            st2[:, TC:TC + n], kT[p][sc][HD:128, lo:lo + SB],
                            qT[p][i][HD:128, tl:tl + n],
                            start=True, stop=True, tile_position=(64, 0))
                        p2 = sp.tile([128, 2 * TC], BF16, tag="pAB", bufs=9,
                                     name="p2")
                        if n == TC:
                            nc.scalar.activation(p2[:], st2[:], AF.Exp, scale=scale)
                        else:
                            st3 = st2[:].rearrange("p (b c) -> p b c", b=2)[:, :, 0:n]
                            p3 = p2[:].rearrange("p (b c) -> p b c", b=2)[:, :, 0:n]
                            nc.scalar.activation(p3, st3, AF.Exp, scale=scale)
                        if mode == "causal" and diag:
                            w_ = s0 + SB - toff
                            for off in (0, TC):
                                nc.gpsimd.affine_select(
                                    out=p2[:, off:off + w_], in_=p2[:, off:off + w_],
                                    compare_op=mybir.AluOpType.is_ge,
                                    fill=0.0, base=toff - s0,
                                    pattern=[[1, w_]], channel_multiplier=-1)
                        elif mode == "general" and cls[s_blk, i] == 2:
                            mmt = sp.tile([SB, TC], BF16, tag="mmask", name="mmt")
                            nc.sync.dma_start(out=mmt[:],
                                              in_=d["mmask"][mixed_idx[(s_blk, i)]])
                            for off in (0, TC):
                                nc.vector.tensor_mul(p2[:, off:off + n],
                                                     p2[:, off:off + n], mmt[:, 0:n])
                        state["p2"][bi] = p2
                    return fn

                def make_pv(bi):
                    s_blk, toff, n, diag = blocks[bi]

                    def fn():
                        if state["ot"] is None:
                            state["ot"] = (
                                psp.tile([HV, TC], F32, tag="ot", bufs=2, name="otA"),
                                psp.tile([HV, TC], F32, tag="ot", bufs=2, name="otB"))
                        otA, otB = state["ot"]
                        p2 = state["p2"].pop(bi)
                        tl = toff - i * TC
                        vv = v_sb[s_blk][:].rearrange("p (h c) -> p h c", c=HV)
                        first, last = bi == 0, bi == len(blocks) - 1
                        nc.tensor.matmul(otA[:, tl:tl + n], vv[:, 2 * p, :],
                                         p2[:, 0:n], start=first, stop=last)
                        nc.tensor.matmul(otB[:, tl:tl + n], vv[:, 2 * p + 1, :],
                                         p2[:, TC:TC + n], start=first, stop=last)
                    return fn

                def epi():
                    otA, otB = state["ot"]
                    fast = (i == NTC - 1 and p == NPAIR - 1)
                    dq = nc.scalar if fast else nc.sync
                    # copy PSUM out up-front so the banks free quickly; use
                    # the (idle) Act engine except in the Act-bound last chunk
                    cpA = mp.tile([HV, TC], F32, tag="ocp", bufs=4, name="cpA")
                    cpB = mp.tile([HV, TC], F32, tag="ocp", bufs=4, name="cpB")
                    if i < NTC - 1:
                        nc.scalar.activation(cpA[:], otA[:], AF.Copy)
                        nc.scalar.activation(cpB[:], otB[:], AF.Copy)
                    else:
                        nc.vector.tensor_copy(cpA[:], otA[:])
                        nc.vector.tensor_copy(cpB[:], otB[:])
                    # both heads' denominators -> partition 0, broadcast, recip
                    dden = mp.tile([1, 2 * TC], F32, tag="dden", name="dden")
                    dq.dma_start(out=dden[:, 0:TC], in_=cpA[HD:HV, :])
                    dq.dma_start(out=dden[:, TC:], in_=cpB[HD:HV, :])
                    rbd = mp.tile([HD, 2 * TC], F32, tag="rbd", bufs=1, name="rbd")
                    if fast:
                        # PE-based broadcast: gpsimd is congested at the tail
                        ones_f = mp.tile([1, HD], F32, tag="onesf", name="onesf")
                        nc.vector.tensor_copy(ones_f[:], ones1_sb[0:1, 0:HD])
                        for hh in range(2):
                            bps = psp.tile([128, TC], F32, tag="b512", bufs=2,
                                           name="bps")
                            nc.tensor.matmul(bps[0:HD, :], ones_f[:],
                                             dden[:, hh * TC:(hh + 1) * TC],
                                             start=True, stop=True)
                            nc.vector.tensor_copy(rbd[:, hh * TC:(hh + 1) * TC],
                                                  bps[0:HD, :])
                    else:
                        nc.gpsimd.partition_broadcast(rbd[:], dden[:])
                    nc.vector.reciprocal_approx_fast(out=rbd[:], in_=rbd[:])
                    nc.vector.tensor_mul(oT[p][i][0:HD, :], cpA[0:HD, :],
                                         rbd[:, 0:TC])
                    stg = mp.tile([HD, TC], BF16, tag="stg", name="stg")
                    nc.vector.tensor_mul(stg[:], cpB[0:HD, :], rbd[:, TC:])
                    dq.dma_start(out=oT[p][i][HD:128, :], in_=stg[:])

                n = len(blocks)
                return ([make_st(b) for b in range(n)],
                        [make_pv(b) for b in range(n)], epi)

            # ---- prologue: pair-0 q/k only (v(0..3) woven into slots) --
            for nm in ("q", "k"):
                for fn in qk_fills(nm, 0, 0):
                    fn()
            if mode != "causal":
                for sg in range(NSB):
                    for fn in v_fills(sg):
                        fn()

            # ---- flat block stream with LAG ----------------------------
            units = [(i, p) for i in range(NTC) for p in range(NPAIR)]
            stream = []
            epis = {}
            for u, (i, p) in enumerate(units):
                st_fns, pv_fns, epi = build_unit(i, p)
                epis[u] = epi
                nb = len(st_fns)
                for b in range(nb):
                    stream.append((st_fns[b], pv_fns[b], u, b == nb - 1))

            def after_chunk(i):
                if i + 2 < NTC:
                    for kind in ("xv", "xq", "xk"):
                        load_x(kind, i + 2)
                # defer out-projection into the Act-bound last chunk, where
                # the PE otherwise runs dry
                um = 4 * (NTC - 1) + i
                for tt in range(4 * i, 4 * i + 4):
                    for fn in outproj_fills(i, tt):
                        fpush(um, fn)

            nblocks = len(stream)
            wo_issued = [False]
            for k in range(nblocks + LAG):
                if k < nblocks:
                    stf, _, u, _ = stream[k]
                    drain_until(u)
                    stf()
                if k == 2 and not wo_issued[0]:
                    wo_issued[0] = True
                    nc.scalar.dma_start(out=wo_sb[:], in_=d["wo"][:])
                    for kind in ("xv", "xq", "xk"):
                        load_x(kind, 1)
                pump(2 if len(fills) > 20 else 1)
                if mode == "causal" and 4 <= k < 8:
                    for fn in v_fills(k - 4):
                        fn()
                j = k - LAG
                if j >= 0:
                    _, pvf, u, last = stream[j]
                    pvf()
                    if last:
                        epis[u]()
                        i, p = units[u]
                        if p == NPAIR - 1 and i < NTC - 1:
                            after_chunk(i)
                        if u == NTC * NPAIR - 2:
                            # oT[0..2][last] now all written: stage the last
                            # chunk's pair-0..2 partials (pump-only)
                            for tt in range(4 * (NTC - 1), 4 * NTC):
                                for fn in outproj_partial_fills(NTC - 1, tt):
                                    fpush(99, fn)
            while fills:
                fills.pop(0)[2]()
            for tt in range(4 * (NTC - 1), 4 * NTC):
                outproj_tail(NTC - 1, tt)

    nc.compile()
    return nc


def kernel(**inputs):
    query = np.asarray(inputs["query"], np.float32)
    key = np.asarray(inputs["key"], np.float32)
    value = np.asarray(inputs["value"], np.float32)
    mask = np.asarray(inputs["mask"], bool)
    Wq, bq = np.asarray(inputs["Wq"], np.float32), np.asarray(inputs["bq"], np.float32)
    Wk, bk = np.asarray(inputs["Wk"], np.float32), np.asarray(inputs["bk"], np.float32)
    Wv, bv = np.asarray(inputs["Wv"], np.float32), np.asarray(inputs["bv"], np.float32)
    Wo, bo = np.asarray(inputs["Wo"], np.float32), np.asarray(inputs["bo"], np.float32)

    mode, cls, mixed = _classify_blocks(mask)
    global mixed_idx
    if mode == "general":
        mixed_idx = {blk: n for n, blk in enumerate(mixed)}
        n_mixed = len(mixed)
    else:
        mixed_idx, n_mixed = {}, 0

    key_sig = (mode, tuple(cls.ravel()) if cls is not None else None)
    if key_sig not in _cache:
        _cache[key_sig] = _build(mode, cls, n_mixed)
    nc = _cache[key_sig]

    def xswz(x):
        # [T, D] activation -> [128, NTC, CCH, PC] (chunk-contig per partition)
        xT = np.ascontiguousarray(x.T).astype(ml_dtypes.bfloat16)
        return np.ascontiguousarray(
            xT.reshape(CCH, 128, NTC, PC).transpose(1, 2, 0, 3))

    def wswz_qk(W, sl):
        # [DG, D] shard -> transpose -> [128, NPAIR, CCH*128] pair-contig
        WT = np.ascontiguousarray(W[sl, :].T).astype(ml_dtypes.bfloat16)
        return np.ascontiguousarray(
            WT.reshape(CCH, 128, NPAIR, 128).transpose(1, 2, 0, 3).reshape(
                128, NPAIR, CCH * 128))

    in_maps = []
    xs = {}
    for b in range(B):
        xs[("xq", b)] = xswz(query[b])
        xs[("xk", b)] = xswz(key[b])
        xs[("xv", b)] = xswz(value[b])
    for core in range(NCORE):
        b, g = core // 2, core % 2
        sl = slice(g * DG, (g + 1) * DG)
        WvT = np.ascontiguousarray(Wv[sl, :].T).astype(ml_dtypes.bfloat16)
        WoT = np.ascontiguousarray(Wo[:, sl].T).astype(ml_dtypes.bfloat16)
        im = {
            "xq": xs[("xq", b)], "xk": xs[("xk", b)], "xv": xs[("xv", b)],
            "wq": wswz_qk(Wq, sl),
            "wk": wswz_qk(Wk, sl),
            "wv": np.ascontiguousarray(
                WvT.reshape(CCH, 128, DG).transpose(1, 0, 2).reshape(128, CCH * DG)),
            "wo": np.ascontiguousarray(
                WoT.reshape(NPAIR, 128, D).transpose(1, 0, 2).reshape(128, NPAIR * D)),
            "bq": np.ascontiguousarray(bq[sl].reshape(NPAIR, 128).T),
            "bk": np.ascontiguousarray(bk[sl].reshape(NPAIR, 128).T),
            "bv": np.ascontiguousarray(bv[sl])[None, :].astype(ml_dtypes.bfloat16),
            "ones1": np.ones((1, 128), ml_dtypes.bfloat16),
        }
        if n_mixed:
            mm = np.empty((n_mixed, SB, TC), ml_dtypes.bfloat16)
            for n, (s_blk, i) in enumerate(mixed):
                blk = mask[b, i * TC:(i + 1) * TC, s_blk * SB:(s_blk + 1) * SB]
                mm[n] = (~blk.T).astype(np.float32)
            im["mmask"] = mm
        in_maps.append(im)

    r = run_bass_kernel_spmd(nc, in_maps, core_ids=list(range(NCORE)))
    last_result["exec_time_ns"] = r.exec_time_ns
    last_result["profile_json"] = getattr(r, "profile_json", None)
    last_result["instructions_and_trace"] = getattr(r, "instructions_and_trace", None)
    out = np.empty((B, T, D), np.float32)
    for b in range(B):
        out[b] = r.results[2 * b]["out"] + r.results[2 * b + 1]["out"]
    out += bo[None, None, :]
    return out
